# revision 6
# baseline (speedup 1.0000x reference)
"""Trainium2 Bass kernel for nn_Critic (MLP value function + GAE).

Sharding: batch B=2048 split across 8 NeuronCores (256 each). MLP params
replicated. The time recurrence (reverse GAE scan) is independent per batch
element, so no cross-core communication.

v3 strategy:
  - Single-pass bf16 matmuls everywhere (fp32 PSUM accumulate). Measured
    numpy emulation gives rel err ~5e-3 vs the 2e-2 gate.
  - states are transposed to feature-major bf16 on the HOST, so the kernel
    does zero PE transposes and zero hi/lo splits.
  - Column order is b-major with reversed time per batch segment:
    col = b*17 + r, r = 16-t. The MLP is row-independent so any column
    permutation works; this one makes the GAE a per-partition scan.
  - Work is streamed in chunks of N=512 columns (8x512 + 1x256): matmul
    free dim 512 = one PSUM bank, near-peak PE streaming.
  - All inputs are host-packed p-major so every SBUF tile loads with ONE
    large DMA (the v2 trace showed 164 small DMAs cost ~600ns of issue
    time each and stalled the PE for ~35us at start). The first chunk's
    states + W0 are split into 1MB pieces across both HWDGE queues so the
    PE can start after ~3us.
  - A few warm-up matmuls on zeroed tiles run during the initial DMA wait
    so the PE HAM clock-gate is at 2.4GHz when real work lands (v2 paid
    ~21us of cold-clock matmuls).
  - value head: Wo is the stationary operand ([128,1] slices) so values
    land in PSUM [1, N]; rows are appended into value_row [1, 4352] and
    reshaped once via a SBUF->SBUF DMA into valP [128, 34] (2 batch
    segments of 17 per partition).
  - GAE: a handful of [128,16] VectorE ops + tensor_tensor_scan per
    segment half. disc/dl (elementwise scalings of `cont`) and all
    reversals/permutations are host-side input prep.
"""

import sys

sys.path.insert(0, "/opt/trn_rl_repo")

import numpy as np

T, B, D, H = 16, 2048, 2048, 1024
NCORES = 8
BC = B // NCORES  # 256 batch per core
TP1 = T + 1
TOT = TP1 * BC  # 4352 MLP rows per core
DISCOUNT, LAMBDA = 0.99, 0.95
P = 128
KD = D // P  # 16 k-tiles for layer 0
KH = H // P  # 8 k-tiles for layers 1,2,out
MH = H // P  # 8 m-tiles of hidden units
CHUNKS = [512] * 8 + [256]  # sum = 4352
SEG = TP1  # 17 values per batch segment
NWARM = 12  # warm-up matmuls

_NC_CACHE = None


def _build():
    import concourse.bacc as bacc
    import concourse.mybir as mybir
    from concourse.tile import TileContext

    F32 = mybir.dt.float32
    BF16 = mybir.dt.bfloat16
    ALU = mybir.AluOpType
    ACTF = mybir.ActivationFunctionType

    nc = bacc.Bacc(None, target_bir_lowering=False, debug=False)

    # statesT: per chunk j (n cols), p-major halves: [2, 128, 8, n];
    # last chunk (n=256) is one [128, 16, n] block. Flattened to rows of 1024.
    statesT_h = nc.declare_dram_parameter("statesT", [D * TOT // 1024, 1024], BF16, isOutput=False)
    # W0: [4 parts, 128, 4, 1024] part/p-major; W1/W2: [128, 8, 1024] p-major.
    w0_h = nc.declare_dram_parameter("W0t", [D, H], BF16, isOutput=False)
    w1_h = nc.declare_dram_parameter("W1t", [H, H], BF16, isOutput=False)
    w2_h = nc.declare_dram_parameter("W2t", [H, H], BF16, isOutput=False)
    wo_h = nc.declare_dram_parameter("WoP", [P, KH], BF16, isOutput=False)
    bias_h = nc.declare_dram_parameter("biasP", [P, 3 * MH], F32, isOutput=False)
    bo_h = nc.declare_dram_parameter("bo", [1, 1], F32, isOutput=False)
    gae_h = nc.declare_dram_parameter("gaeP", [P, 6 * T], F32, isOutput=False)
    ret_h = nc.declare_dram_parameter("retP", [P, 2 * T], F32, isOutput=True)
    val_h = nc.declare_dram_parameter("valP", [P, 2 * T], F32, isOutput=True)

    with TileContext(nc) as tc:
        with (
            tc.tile_pool(name="wpool", bufs=1) as wpool,
            tc.tile_pool(name="stpool", bufs=1) as stpool,
            tc.tile_pool(name="hpool", bufs=1) as hpool,
            tc.tile_pool(name="tmp", bufs=3) as tmppool,
            tc.tile_pool(name="gae", bufs=1) as gaepool,
            tc.tile_pool(name="psA", bufs=4, space="PSUM") as psApool,
            tc.tile_pool(name="psV", bufs=2, space="PSUM") as psVpool,
            tc.tile_pool(name="psW", bufs=1, space="PSUM") as psWpool,
        ):
            # ---- PE warm-up on zeroed tiles (overlaps the first DMAs) ----
            zw = wpool.tile([P, P], BF16, name="zw", tag="zw")
            nc.vector.memset(zw[:], 0.0)
            zx = wpool.tile([P, 512], BF16, name="zx", tag="zx")
            nc.vector.memset(zx[:], 0.0)
            zp = psWpool.tile([P, 512], F32, name="zp", tag="zp")
            for _ in range(NWARM):
                nc.tensor.matmul(
                    zp[:], lhsT=zw[:], rhs=zx[:], start=True, stop=True,
                    skip_group_check=True,
                )

            # ---- weights / constants (one big DMA per tensor) ----
            w0all = wpool.tile([P, KD * H], BF16, name="w0all", tag="w0all")
            for q in range(4):
                nc.scalar.dma_start(
                    out=w0all[:, q * 4 * H : (q + 1) * 4 * H],
                    in_=w0_h[q * 512 : (q + 1) * 512, :],
                )
            w1all = wpool.tile([P, KH * H], BF16, name="w1all", tag="w1all")
            nc.scalar.dma_start(out=w1all[:], in_=w1_h[:])
            w2all = wpool.tile([P, KH * H], BF16, name="w2all", tag="w2all")
            nc.scalar.dma_start(out=w2all[:], in_=w2_h[:])
            wall = (w0all, w1all, w2all)

            wosb = wpool.tile([P, KH], BF16, name="wosb", tag="wosb")
            biasP = wpool.tile([P, 3 * MH], F32, name="biasP", tag="biasP")
            bosb = wpool.tile([1, 1], F32, name="bosb", tag="bosb")
            gaesb = gaepool.tile([P, 6 * T], F32, name="gaesb", tag="gaesb")
            rewsb = gaesb[:, 0 : 2 * T]
            discsb = gaesb[:, 2 * T : 4 * T]
            dlsb = gaesb[:, 4 * T : 6 * T]

            def load_consts():
                # emitted after chunk 0's states DMA so they don't delay it
                nc.sync.dma_start(out=wosb[:], in_=wo_h[:])
                nc.sync.dma_start(out=biasP[:], in_=bias_h[:])
                nc.sync.dma_start(out=bosb[:], in_=bo_h[:])
                nc.sync.dma_start(out=gaesb[:], in_=gae_h[:])

            value_row = gaepool.tile([1, TOT], F32, name="value_row", tag="value_row")
            valP = gaepool.tile([P, 2 * SEG], F32, name="valPsb", tag="valPsb")
            dtt = gaepool.tile([P, 2 * T], F32, name="dtt", tag="dtt")
            adv = gaepool.tile([P, 2 * T], F32, name="adv", tag="adv")
            retP = gaepool.tile([P, 2 * T], F32, name="retP", tag="retP")

            ALUc = ALU

            def gae_half(p0, p1):
                # GAE for partitions [p0, p1): value_row cols [p0*34, p1*34).
                pp = slice(p0, p1)
                nc.sync.dma_start(
                    out=valP[pp, :], in_=value_row[0:1, p0 * 2 * SEG : p1 * 2 * SEG]
                )
                for s in range(2):
                    ss = slice(s * T, (s + 1) * T)
                    vnext = valP[pp, s * SEG : s * SEG + T]
                    vcur = valP[pp, s * SEG + 1 : s * SEG + 1 + T]
                    nc.vector.tensor_mul(dtt[pp, ss], discsb[pp, ss], vnext)
                    nc.vector.tensor_add(dtt[pp, ss], dtt[pp, ss], rewsb[pp, ss])
                    nc.vector.tensor_sub(dtt[pp, ss], dtt[pp, ss], vcur)
                    nc.vector.tensor_tensor_scan(
                        adv[pp, ss], dlsb[pp, ss], dtt[pp, ss], 0.0, ALUc.mult, ALUc.add
                    )
                    nc.vector.tensor_add(retP[pp, ss], adv[pp, ss], vcur)
                    nc.sync.dma_start(out=val_h[pp, ss], in_=vcur)
                nc.sync.dma_start(out=ret_h[pp, :], in_=retP[pp, :])

            # ---- streamed MLP over column chunks ----
            c0 = 0
            row0 = 0
            for ci, n in enumerate(CHUNKS):
                st_all = stpool.tile([P, KD * n], BF16, name="st", tag="st", bufs=2)
                nrows = KD * P * n // 1024  # 1024 (n=512) or 512 (n=256)
                ndma = 2 if n == 512 else 1
                for h in range(ndma):
                    hr = nrows // ndma
                    nc.sync.dma_start(
                        out=st_all[:, h * (KD * n // ndma) : (h + 1) * (KD * n // ndma)],
                        in_=statesT_h[row0 + h * hr : row0 + (h + 1) * hr, :],
                    )
                row0 += nrows
                if ci == 0:
                    load_consts()

                hs = []
                for li, nk in ((0, KD), (1, KH), (2, KH)):
                    rhs_src = st_all if li == 0 else hs[-1]
                    hout = hpool.tile([P, MH * n], BF16, name=f"h{li}", tag=f"h{li}", bufs=2)
                    for m in range(MH):
                        psm = psApool.tile([P, n], F32, name="psm", tag="psm")
                        for k in range(nk):
                            nc.tensor.matmul(
                                psm[:],
                                lhsT=wall[li][:, k * H + m * P : k * H + (m + 1) * P],
                                rhs=rhs_src[:, k * n : (k + 1) * n],
                                start=(k == 0),
                                stop=(k == nk - 1),
                                skip_group_check=True,
                            )
                        # ELU(z+b) = min(exp(z+b)-1, relu(z+b))
                        bcol = biasP[:, li * MH + m : li * MH + m + 1]
                        e = tmppool.tile([P, n], F32, name="e", tag="e")
                        nc.scalar.activation(e[:], psm[:], ACTF.Exp, bias=bcol)
                        rl = tmppool.tile([P, n], F32, name="rl", tag="rl")
                        nc.vector.tensor_scalar(
                            rl[:], psm[:], bcol, 0.0, ALU.add, ALU.max
                        )
                        nc.vector.scalar_tensor_tensor(
                            hout[:, m * n : (m + 1) * n],
                            e[:],
                            1.0,
                            rl[:],
                            ALU.subtract,
                            ALU.min,
                        )
                    hs.append(hout)

                # value head: Wo stationary [128,1] -> value lands [1, n]
                pv = psVpool.tile([1, n], F32, name="pv", tag="pv")
                for k in range(KH):
                    nc.tensor.matmul(
                        pv[:],
                        lhsT=wosb[:, k : k + 1],
                        rhs=hs[2][:, k * n : (k + 1) * n],
                        start=(k == 0),
                        stop=(k == KH - 1),
                        skip_group_check=True,
                    )
                nc.vector.tensor_scalar_add(
                    value_row[0:1, c0 : c0 + n], pv[:], bosb[0:1, 0:1]
                )
                c0 += n
                # GAE for partitions 0..63 (value_row cols < 2176) can run
                # as soon as chunk 4 (c0 = 2560) is done; it hides under
                # chunks 5-8. The rest runs in the tail.
                if ci == 4:
                    gae_half(0, 64)
            gae_half(64, P)

    nc.compile()
    return nc


def _get_nc():
    global _NC_CACHE
    if _NC_CACHE is None:
        _NC_CACHE = _build()
    return _NC_CACHE


def _pack_pmajor(w, nk):
    # [nk*128, cols] -> p-major [128, nk, cols] flattened back to same shape
    cols = w.shape[1]
    return np.ascontiguousarray(
        w.reshape(nk, P, cols).transpose(1, 0, 2).reshape(nk * P, cols)
    )


def _make_in_maps(inputs):
    import ml_dtypes

    BF = ml_dtypes.bfloat16
    states = np.asarray(inputs["states"], dtype=np.float32)
    reward = np.asarray(inputs["reward"], dtype=np.float32)
    cont = np.asarray(inputs["cont"], dtype=np.float32)

    # Feature-major states, b-major columns with reversed time:
    # full[d, b, r] = states[16-r, b, d] in bf16.
    st_bf = states.astype(BF)
    full = np.ascontiguousarray(st_bf[::-1].transpose(2, 1, 0))  # [D, B, TP1]

    W0 = np.asarray(inputs["W0"], np.float32).astype(BF)
    W1 = np.asarray(inputs["W1"], np.float32).astype(BF)
    W2 = np.asarray(inputs["W2"], np.float32).astype(BF)
    # W0: [4 parts, 128, 4, 1024] part-major then p-major
    W0t = np.ascontiguousarray(
        W0.reshape(4, 4, P, H).transpose(0, 2, 1, 3).reshape(D, H)
    )
    W1t = _pack_pmajor(W1, KH)
    W2t = _pack_pmajor(W2, KH)
    WoP = np.ascontiguousarray(
        np.asarray(inputs["Wo"], np.float32).astype(BF).reshape(KH, P).T
    )
    b3 = np.stack(
        [np.asarray(inputs[k], np.float32) for k in ("b0", "b1", "b2")]
    )  # [3, 1024]
    biasP = np.ascontiguousarray(b3.reshape(3, MH, P).transpose(2, 0, 1).reshape(P, 3 * MH))
    bo = np.ascontiguousarray(np.asarray(inputs["bo"], np.float32).reshape(1, 1))

    in_maps = []
    for c in range(NCORES):
        sl = slice(c * BC, (c + 1) * BC)
        # statesT for this core: [D, 4352] b-major/rev-t columns, then
        # per chunk: halves x [128, 8|16, n] p-major, flattened.
        stT = full[:, sl, :].reshape(D, TOT)
        blocks = []
        c0 = 0
        for n in CHUNKS:
            blk = stT[:, c0 : c0 + n].reshape(KD, P, n)  # [k, p, n]
            ndma = 2 if n == 512 else 1
            kk = KD // ndma
            blocks.append(
                np.ascontiguousarray(
                    blk.reshape(ndma, kk, P, n).transpose(0, 2, 1, 3)
                ).reshape(-1)
            )
            c0 += n
        statesT = np.concatenate(blocks).reshape(D * TOT // 1024, 1024)

        # rewP[p, s*16+j] = reward[15-j, 2p+s]; disc uses cont[16-j].
        rr = reward[::-1, sl]  # [T, BC], rr[j] = reward[15-j]
        cc = cont[1:][::-1, sl]  # [T, BC], cc[j] = cont[16-j]
        rewP = rr.T.reshape(P, 2 * T)
        discP = (DISCOUNT * cc).T.reshape(P, 2 * T)
        dlP = (DISCOUNT * LAMBDA * cc).T.reshape(P, 2 * T)
        gaeP = np.ascontiguousarray(np.concatenate([rewP, discP, dlP], axis=1))
        in_maps.append(
            {
                "statesT": statesT,
                "W0t": W0t,
                "W1t": W1t,
                "W2t": W2t,
                "WoP": WoP,
                "biasP": biasP,
                "bo": bo,
                "gaeP": gaeP,
            }
        )
    return in_maps


def _run(inputs, trace=False):
    from concourse.bass_utils import run_bass_kernel_spmd

    nc = _get_nc()
    in_maps = _make_in_maps(inputs)
    bkr = run_bass_kernel_spmd(nc, in_maps, list(range(NCORES)), trace=trace)
    ret = np.empty((T, B), np.float32)
    val = np.empty((T, B), np.float32)
    for c in range(NCORES):
        sl = slice(c * BC, (c + 1) * BC)
        # retP[p, s*16+j] -> ret[15-j, 2p+s]
        rp = bkr.results[c]["retP"].reshape(P, 2, T)[:, :, ::-1]  # [p, s, t]
        vp = bkr.results[c]["valP"].reshape(P, 2, T)[:, :, ::-1]
        ret[:, sl] = rp.transpose(2, 0, 1).reshape(T, BC)
        val[:, sl] = vp.transpose(2, 0, 1).reshape(T, BC)
    return (ret, val), bkr


def kernel(**inputs):
    out, _ = _run(inputs, trace=False)
    return out


# revision 11
# speedup vs baseline: 1.0036x; 1.0036x over previous
"""Trainium2 Bass kernel for nn_Critic (MLP value function + GAE).

Sharding: batch B=2048 split across 8 NeuronCores (256 each). MLP params
replicated. The time recurrence (reverse GAE scan) is independent per batch
element, so no cross-core communication.

v3 strategy:
  - Single-pass bf16 matmuls everywhere (fp32 PSUM accumulate). Measured
    numpy emulation gives rel err ~5e-3 vs the 2e-2 gate.
  - states are transposed to feature-major bf16 on the HOST, so the kernel
    does zero PE transposes and zero hi/lo splits.
  - Column order is b-major with reversed time per batch segment:
    col = b*17 + r, r = 16-t. The MLP is row-independent so any column
    permutation works; this one makes the GAE a per-partition scan.
  - Work is streamed in chunks of N=512 columns (8x512 + 1x256): matmul
    free dim 512 = one PSUM bank, near-peak PE streaming.
  - All inputs are host-packed p-major so every SBUF tile loads with ONE
    large DMA (the v2 trace showed 164 small DMAs cost ~600ns of issue
    time each and stalled the PE for ~35us at start). The first chunk's
    states + W0 are split into 1MB pieces across both HWDGE queues so the
    PE can start after ~3us.
  - A few warm-up matmuls on zeroed tiles run during the initial DMA wait
    so the PE HAM clock-gate is at 2.4GHz when real work lands (v2 paid
    ~21us of cold-clock matmuls).
  - value head: Wo is the stationary operand ([128,1] slices) so values
    land in PSUM [1, N]; rows are appended into value_row [1, 4352] and
    reshaped once via a SBUF->SBUF DMA into valP [128, 34] (2 batch
    segments of 17 per partition).
  - GAE: a handful of [128,16] VectorE ops + tensor_tensor_scan per
    segment half. disc/dl (elementwise scalings of `cont`) and all
    reversals/permutations are host-side input prep.
"""

import sys

sys.path.insert(0, "/opt/trn_rl_repo")

import numpy as np

T, B, D, H = 16, 2048, 2048, 1024
NCORES = 8
BC = B // NCORES  # 256 batch per core
TP1 = T + 1
TOT = TP1 * BC  # 4352 MLP rows per core
DISCOUNT, LAMBDA = 0.99, 0.95
P = 128
KD = D // P  # 16 k-tiles for layer 0
KH = H // P  # 8 k-tiles for layers 1,2,out
MH = H // P  # 8 m-tiles of hidden units
CHUNKS = [512] * 8 + [256]  # sum = 4352
SEG = TP1  # 17 values per batch segment
NWARM = 12  # warm-up matmuls

_NC_CACHE = None


def _build():
    import concourse.bacc as bacc
    import concourse.mybir as mybir
    from concourse.tile import TileContext

    F32 = mybir.dt.float32
    BF16 = mybir.dt.bfloat16
    ALU = mybir.AluOpType
    ACTF = mybir.ActivationFunctionType

    nc = bacc.Bacc(None, target_bir_lowering=False, debug=False)

    # statesT: per chunk j (n cols), p-major halves: [2, 128, 8, n];
    # last chunk (n=256) is one [128, 16, n] block. Flattened to rows of 1024.
    statesT_h = nc.declare_dram_parameter("statesT", [D * TOT // 1024, 1024], BF16, isOutput=False)
    # W0: [4 parts, 128, 4, 1024] part/p-major; W1/W2: [128, 8, 1024] p-major.
    w0_h = nc.declare_dram_parameter("W0t", [D, H], BF16, isOutput=False)
    w1_h = nc.declare_dram_parameter("W1t", [H, H], BF16, isOutput=False)
    w2_h = nc.declare_dram_parameter("W2t", [H, H], BF16, isOutput=False)
    wo_h = nc.declare_dram_parameter("WoP", [P, KH], BF16, isOutput=False)
    bias_h = nc.declare_dram_parameter("biasP", [P, 3 * MH], F32, isOutput=False)
    bo_h = nc.declare_dram_parameter("bo", [1, 1], F32, isOutput=False)
    gae_h = nc.declare_dram_parameter("gaeP", [P, 6 * T], F32, isOutput=False)
    ret_h = nc.declare_dram_parameter("retP", [P, 2 * T], F32, isOutput=True)
    val_h = nc.declare_dram_parameter("valP", [P, 2 * T], F32, isOutput=True)

    with TileContext(nc) as tc:
        with (
            tc.tile_pool(name="wpool", bufs=1) as wpool,
            tc.tile_pool(name="stpool", bufs=1) as stpool,
            tc.tile_pool(name="hpool", bufs=1) as hpool,
            tc.tile_pool(name="tmp", bufs=3) as tmppool,
            tc.tile_pool(name="gae", bufs=1) as gaepool,
            tc.tile_pool(name="psA", bufs=4, space="PSUM") as psApool,
            tc.tile_pool(name="psV", bufs=2, space="PSUM") as psVpool,
            tc.tile_pool(name="psW", bufs=1, space="PSUM") as psWpool,
        ):
            # ---- PE warm-up on zeroed tiles (overlaps the first DMAs) ----
            zw = wpool.tile([P, P], BF16, name="zw", tag="zw")
            nc.vector.memset(zw[:], 0.0)
            zx = wpool.tile([P, 512], BF16, name="zx", tag="zx")
            nc.vector.memset(zx[:], 0.0)
            zp = psWpool.tile([P, 512], F32, name="zp", tag="zp")
            for _ in range(NWARM):
                nc.tensor.matmul(
                    zp[:], lhsT=zw[:], rhs=zx[:], start=True, stop=True,
                    skip_group_check=True,
                )

            # ---- weights / constants ----
            # biasP first: the first ELU needs it, and ELUs recycle PSUM.
            biasP = wpool.tile([P, 3 * MH], F32, name="biasP", tag="biasP")
            nc.sync.dma_start(out=biasP[:], in_=bias_h[:])
            # W0 as 8 separate 512KB piece-tiles (2 k-tiles each) so chunk-0
            # matmuls depend only on the piece they read, not the whole 4MB.
            w0p = []
            for q in range(KD // 2):
                wt = wpool.tile([P, 2 * H], BF16, name=f"w0p{q}", tag=f"w0p{q}")
                nc.scalar.dma_start(out=wt[:], in_=w0_h[q * 256 : (q + 1) * 256, :])
                w0p.append(wt)

            def w0slice(k, m):
                return w0p[k // 2][:, (k % 2) * H + m * P : (k % 2) * H + (m + 1) * P]

            w1all = wpool.tile([P, KH * H], BF16, name="w1all", tag="w1all")
            nc.scalar.dma_start(out=w1all[:], in_=w1_h[:])
            w2all = wpool.tile([P, KH * H], BF16, name="w2all", tag="w2all")
            nc.scalar.dma_start(out=w2all[:], in_=w2_h[:])
            wall = (None, w1all, w2all)

            wosb = wpool.tile([P, KH], BF16, name="wosb", tag="wosb")
            bosb = wpool.tile([1, 1], F32, name="bosb", tag="bosb")
            gaesb = gaepool.tile([P, 6 * T], F32, name="gaesb", tag="gaesb")
            rewsb = gaesb[:, 0 : 2 * T]
            discsb = gaesb[:, 2 * T : 4 * T]
            dlsb = gaesb[:, 4 * T : 6 * T]

            def load_consts():
                # emitted after chunk 0's states DMA so they don't delay it
                nc.sync.dma_start(out=wosb[:], in_=wo_h[:])
                nc.sync.dma_start(out=bosb[:], in_=bo_h[:])
                nc.sync.dma_start(out=gaesb[:], in_=gae_h[:])

            value_row = gaepool.tile([1, TOT], F32, name="value_row", tag="value_row")
            valP = gaepool.tile([P, 2 * SEG], F32, name="valPsb", tag="valPsb")
            dtt = gaepool.tile([P, 2 * T], F32, name="dtt", tag="dtt")
            adv = gaepool.tile([P, 2 * T], F32, name="adv", tag="adv")
            retP = gaepool.tile([P, 2 * T], F32, name="retP", tag="retP")

            ALUc = ALU

            def gae_half(p0, p1):
                # GAE for partitions [p0, p1): value_row cols [p0*34, p1*34).
                pp = slice(p0, p1)
                nc.sync.dma_start(
                    out=valP[pp, :], in_=value_row[0:1, p0 * 2 * SEG : p1 * 2 * SEG]
                )
                for s in range(2):
                    ss = slice(s * T, (s + 1) * T)
                    vnext = valP[pp, s * SEG : s * SEG + T]
                    vcur = valP[pp, s * SEG + 1 : s * SEG + 1 + T]
                    nc.vector.tensor_mul(dtt[pp, ss], discsb[pp, ss], vnext)
                    nc.vector.tensor_add(dtt[pp, ss], dtt[pp, ss], rewsb[pp, ss])
                    nc.vector.tensor_sub(dtt[pp, ss], dtt[pp, ss], vcur)
                    nc.vector.tensor_tensor_scan(
                        adv[pp, ss], dlsb[pp, ss], dtt[pp, ss], 0.0, ALUc.mult, ALUc.add
                    )
                    nc.vector.tensor_add(retP[pp, ss], adv[pp, ss], vcur)
                    nc.sync.dma_start(out=val_h[pp, ss], in_=vcur)
                nc.sync.dma_start(out=ret_h[pp, :], in_=retP[pp, :])

            # ---- streamed MLP over column chunks ----
            def elu(psm, li, m, hout, n):
                # ELU(z+b) = min(exp(z+b)-1, relu(z+b))
                bcol = biasP[:, li * MH + m : li * MH + m + 1]
                e = tmppool.tile([P, n], F32, name="e", tag="e")
                nc.scalar.activation(e[:], psm[:], ACTF.Exp, bias=bcol)
                rl = tmppool.tile([P, n], F32, name="rl", tag="rl")
                nc.vector.tensor_scalar(rl[:], psm[:], bcol, 0.0, ALU.add, ALU.max)
                nc.vector.scalar_tensor_tensor(
                    hout[:, m * n : (m + 1) * n], e[:], 1.0, rl[:],
                    ALU.subtract, ALU.min,
                )

            c0 = 0
            row0 = 0
            for ci, n in enumerate(CHUNKS):
                nrows = KD * P * n // 1024  # 1024 (n=512) or 512 (n=256)
                if ci == 0:
                    # chunk 0: four separate quarter-tiles (4 k-tiles each)
                    # so matmuls can start as each 512KB piece lands.
                    stq = []
                    for qi in range(4):
                        sq = stpool.tile([P, 4 * n], BF16, name=f"st0q{qi}", tag=f"st0q{qi}")
                        hr = nrows // 4
                        nc.sync.dma_start(
                            out=sq[:],
                            in_=statesT_h[row0 + qi * hr : row0 + (qi + 1) * hr, :],
                        )
                        stq.append(sq)
                    load_consts()

                    def st0slice(k):
                        return stq[k // 4][:, (k % 4) * n : (k % 4 + 1) * n]

                else:
                    st_all = stpool.tile([P, KD * n], BF16, name="st", tag="st", bufs=2)
                    nc.sync.dma_start(
                        out=st_all[:], in_=statesT_h[row0 : row0 + nrows, :]
                    )
                row0 += nrows

                hs = []
                for li, nk in ((0, KD), (1, KH), (2, KH)):
                    rhs_src = hs[-1] if li else None
                    hout = hpool.tile([P, MH * n], BF16, name=f"h{li}", tag=f"h{li}", bufs=2)
                    if li == 0 and ci == 0:
                        # pass A: k-outer for m 0..3, consuming DMA pieces
                        # as they arrive (PE never waits for the full 6MB).
                        psms = [
                            psApool.tile([P, n], F32, name="psm", tag="psm")
                            for _ in range(4)
                        ]
                        for k in range(KD):
                            for mi, psm in enumerate(psms):
                                nc.tensor.matmul(
                                    psm[:], lhsT=w0slice(k, mi), rhs=st0slice(k),
                                    start=(k == 0), stop=(k == KD - 1),
                                    skip_group_check=True,
                                )
                        for mi, psm in enumerate(psms):
                            elu(psm, 0, mi, hout, n)
                        # pass B: m-outer for m 4..7 (everything resident now)
                        for m in range(4, MH):
                            psm = psApool.tile([P, n], F32, name="psm", tag="psm")
                            for k in range(KD):
                                nc.tensor.matmul(
                                    psm[:], lhsT=w0slice(k, m), rhs=st0slice(k),
                                    start=(k == 0), stop=(k == KD - 1),
                                    skip_group_check=True,
                                )
                            elu(psm, 0, m, hout, n)
                        hs.append(hout)
                        continue
                    for m in range(MH):
                        psm = psApool.tile([P, n], F32, name="psm", tag="psm")
                        for k in range(nk):
                            if li == 0:
                                lhsT = w0slice(k, m)
                                rhs = st_all[:, k * n : (k + 1) * n]
                            else:
                                lhsT = wall[li][:, k * H + m * P : k * H + (m + 1) * P]
                                rhs = rhs_src[:, k * n : (k + 1) * n]
                            nc.tensor.matmul(
                                psm[:], lhsT=lhsT, rhs=rhs,
                                start=(k == 0), stop=(k == nk - 1),
                                skip_group_check=True,
                            )
                        elu(psm, li, m, hout, n)
                    hs.append(hout)

                # value head: Wo stationary [128,1] -> value lands [1, n]
                pv = psVpool.tile([1, n], F32, name="pv", tag="pv")
                for k in range(KH):
                    nc.tensor.matmul(
                        pv[:],
                        lhsT=wosb[:, k : k + 1],
                        rhs=hs[2][:, k * n : (k + 1) * n],
                        start=(k == 0),
                        stop=(k == KH - 1),
                        skip_group_check=True,
                    )
                nc.vector.tensor_scalar_add(
                    value_row[0:1, c0 : c0 + n], pv[:], bosb[0:1, 0:1]
                )
                c0 += n
                # GAE for partitions 0..63 (value_row cols < 2176) can run
                # as soon as chunk 4 (c0 = 2560) is done; it hides under
                # chunks 5-8. The rest runs in the tail.
                if ci == 4:
                    gae_half(0, 64)
            gae_half(64, P)

    nc.compile()
    return nc


def _get_nc():
    global _NC_CACHE
    if _NC_CACHE is None:
        _NC_CACHE = _build()
    return _NC_CACHE


def _pack_pmajor(w, nk):
    # [nk*128, cols] -> p-major [128, nk, cols] flattened back to same shape
    cols = w.shape[1]
    return np.ascontiguousarray(
        w.reshape(nk, P, cols).transpose(1, 0, 2).reshape(nk * P, cols)
    )


def _make_in_maps(inputs):
    import ml_dtypes

    BF = ml_dtypes.bfloat16
    states = np.asarray(inputs["states"], dtype=np.float32)
    reward = np.asarray(inputs["reward"], dtype=np.float32)
    cont = np.asarray(inputs["cont"], dtype=np.float32)

    # Feature-major states, b-major columns with reversed time:
    # full[d, b, r] = states[16-r, b, d] in bf16.
    st_bf = states.astype(BF)
    full = np.ascontiguousarray(st_bf[::-1].transpose(2, 1, 0))  # [D, B, TP1]

    W0 = np.asarray(inputs["W0"], np.float32).astype(BF)
    W1 = np.asarray(inputs["W1"], np.float32).astype(BF)
    W2 = np.asarray(inputs["W2"], np.float32).astype(BF)
    # W0: [8 parts, 128, 2, 1024] part-major then p-major
    W0t = np.ascontiguousarray(
        W0.reshape(8, 2, P, H).transpose(0, 2, 1, 3).reshape(D, H)
    )
    W1t = _pack_pmajor(W1, KH)
    W2t = _pack_pmajor(W2, KH)
    WoP = np.ascontiguousarray(
        np.asarray(inputs["Wo"], np.float32).astype(BF).reshape(KH, P).T
    )
    b3 = np.stack(
        [np.asarray(inputs[k], np.float32) for k in ("b0", "b1", "b2")]
    )  # [3, 1024]
    biasP = np.ascontiguousarray(b3.reshape(3, MH, P).transpose(2, 0, 1).reshape(P, 3 * MH))
    bo = np.ascontiguousarray(np.asarray(inputs["bo"], np.float32).reshape(1, 1))

    in_maps = []
    for c in range(NCORES):
        sl = slice(c * BC, (c + 1) * BC)
        # statesT for this core: [D, 4352] b-major/rev-t columns, then
        # per chunk: halves x [128, 8|16, n] p-major, flattened.
        stT = full[:, sl, :].reshape(D, TOT)
        blocks = []
        c0 = 0
        for ci, n in enumerate(CHUNKS):
            blk = stT[:, c0 : c0 + n].reshape(KD, P, n)  # [k, p, n]
            ndma = 4 if ci == 0 else 1
            kk = KD // ndma
            blocks.append(
                np.ascontiguousarray(
                    blk.reshape(ndma, kk, P, n).transpose(0, 2, 1, 3)
                ).reshape(-1)
            )
            c0 += n
        statesT = np.concatenate(blocks).reshape(D * TOT // 1024, 1024)

        # rewP[p, s*16+j] = reward[15-j, 2p+s]; disc uses cont[16-j].
        rr = reward[::-1, sl]  # [T, BC], rr[j] = reward[15-j]
        cc = cont[1:][::-1, sl]  # [T, BC], cc[j] = cont[16-j]
        rewP = rr.T.reshape(P, 2 * T)
        discP = (DISCOUNT * cc).T.reshape(P, 2 * T)
        dlP = (DISCOUNT * LAMBDA * cc).T.reshape(P, 2 * T)
        gaeP = np.ascontiguousarray(np.concatenate([rewP, discP, dlP], axis=1))
        in_maps.append(
            {
                "statesT": statesT,
                "W0t": W0t,
                "W1t": W1t,
                "W2t": W2t,
                "WoP": WoP,
                "biasP": biasP,
                "bo": bo,
                "gaeP": gaeP,
            }
        )
    return in_maps


def _run(inputs, trace=False):
    from concourse.bass_utils import run_bass_kernel_spmd

    nc = _get_nc()
    in_maps = _make_in_maps(inputs)
    bkr = run_bass_kernel_spmd(nc, in_maps, list(range(NCORES)), trace=trace)
    ret = np.empty((T, B), np.float32)
    val = np.empty((T, B), np.float32)
    for c in range(NCORES):
        sl = slice(c * BC, (c + 1) * BC)
        # retP[p, s*16+j] -> ret[15-j, 2p+s]
        rp = bkr.results[c]["retP"].reshape(P, 2, T)[:, :, ::-1]  # [p, s, t]
        vp = bkr.results[c]["valP"].reshape(P, 2, T)[:, :, ::-1]
        ret[:, sl] = rp.transpose(2, 0, 1).reshape(T, BC)
        val[:, sl] = vp.transpose(2, 0, 1).reshape(T, BC)
    return (ret, val), bkr


def kernel(**inputs):
    out, _ = _run(inputs, trace=False)
    return out


# revision 16
# speedup vs baseline: 1.0248x; 1.0211x over previous
"""Trainium2 Bass kernel for nn_Critic (MLP value function + GAE).

Sharding: batch B=2048 split across 8 NeuronCores (256 each). MLP params
replicated. The time recurrence (reverse GAE scan) is independent per batch
element, so no cross-core communication.

v3 strategy:
  - Single-pass bf16 matmuls everywhere (fp32 PSUM accumulate). Measured
    numpy emulation gives rel err ~5e-3 vs the 2e-2 gate.
  - states are transposed to feature-major bf16 on the HOST, so the kernel
    does zero PE transposes and zero hi/lo splits.
  - Column order is b-major with reversed time per batch segment:
    col = b*17 + r, r = 16-t. The MLP is row-independent so any column
    permutation works; this one makes the GAE a per-partition scan.
  - Work is streamed in chunks of N=512 columns (8x512 + 1x256): matmul
    free dim 512 = one PSUM bank, near-peak PE streaming.
  - All inputs are host-packed p-major so every SBUF tile loads with ONE
    large DMA (the v2 trace showed 164 small DMAs cost ~600ns of issue
    time each and stalled the PE for ~35us at start). The first chunk's
    states + W0 are split into 1MB pieces across both HWDGE queues so the
    PE can start after ~3us.
  - A few warm-up matmuls on zeroed tiles run during the initial DMA wait
    so the PE HAM clock-gate is at 2.4GHz when real work lands (v2 paid
    ~21us of cold-clock matmuls).
  - value head: Wo is the stationary operand ([128,1] slices) so values
    land in PSUM [1, N]; rows are appended into value_row [1, 4352] and
    reshaped once via a SBUF->SBUF DMA into valP [128, 34] (2 batch
    segments of 17 per partition).
  - GAE: a handful of [128,16] VectorE ops + tensor_tensor_scan per
    segment half. disc/dl (elementwise scalings of `cont`) and all
    reversals/permutations are host-side input prep.
"""

import sys

sys.path.insert(0, "/opt/trn_rl_repo")

import numpy as np

T, B, D, H = 16, 2048, 2048, 1024
NCORES = 8
BC = B // NCORES  # 256 batch per core
TP1 = T + 1
TOT = TP1 * BC  # 4352 MLP rows per core
DISCOUNT, LAMBDA = 0.99, 0.95
P = 128
KD = D // P  # 16 k-tiles for layer 0
KH = H // P  # 8 k-tiles for layers 1,2,out
MH = H // P  # 8 m-tiles of hidden units
CHUNKS = [512] * 8 + [256]  # sum = 4352
SEG = TP1  # 17 values per batch segment
NWARM = 12  # warm-up matmuls

_NC_CACHE = None


def _build():
    import concourse.bacc as bacc
    import concourse.mybir as mybir
    from concourse.tile import TileContext

    F32 = mybir.dt.float32
    BF16 = mybir.dt.bfloat16
    ALU = mybir.AluOpType
    ACTF = mybir.ActivationFunctionType

    nc = bacc.Bacc(None, target_bir_lowering=False, debug=False)

    # statesT: per chunk j (n cols), p-major halves: [2, 128, 8, n];
    # last chunk (n=256) is one [128, 16, n] block. Flattened to rows of 1024.
    statesT_h = nc.declare_dram_parameter("statesT", [D * TOT // 1024, 1024], BF16, isOutput=False)
    # W0: [4 parts, 128, 4, 1024] part/p-major; W1/W2: [128, 8, 1024] p-major.
    w0_h = nc.declare_dram_parameter("W0t", [D, H], BF16, isOutput=False)
    w1_h = nc.declare_dram_parameter("W1t", [H, H], BF16, isOutput=False)
    w2_h = nc.declare_dram_parameter("W2t", [H, H], BF16, isOutput=False)
    wo_h = nc.declare_dram_parameter("WoP", [P, KH], BF16, isOutput=False)
    bias_h = nc.declare_dram_parameter("biasP", [P, 3 * MH], F32, isOutput=False)
    bo_h = nc.declare_dram_parameter("bo", [1, 1], F32, isOutput=False)
    gae_h = nc.declare_dram_parameter("gaeP", [P, 6 * T], F32, isOutput=False)
    ret_h = nc.declare_dram_parameter("retP", [P, 2 * T], F32, isOutput=True)
    val_h = nc.declare_dram_parameter("valP", [P, 2 * T], F32, isOutput=True)

    with TileContext(nc) as tc:
        with (
            tc.tile_pool(name="wpool", bufs=1) as wpool,
            tc.tile_pool(name="stpool", bufs=1) as stpool,
            tc.tile_pool(name="hpool", bufs=1) as hpool,
            tc.tile_pool(name="tmp", bufs=3) as tmppool,
            tc.tile_pool(name="gae", bufs=1) as gaepool,
            tc.tile_pool(name="psA", bufs=5, space="PSUM") as psApool,
            tc.tile_pool(name="psV", bufs=2, space="PSUM") as psVpool,
            tc.tile_pool(name="psW", bufs=1, space="PSUM") as psWpool,
        ):
            # ---- PE warm-up on zeroed tiles (overlaps the first DMAs) ----
            zw = wpool.tile([P, P], BF16, name="zw", tag="zw")
            nc.vector.memset(zw[:], 0.0)
            zx = wpool.tile([P, 512], BF16, name="zx", tag="zx")
            nc.vector.memset(zx[:], 0.0)
            zp = psWpool.tile([P, 512], F32, name="zp", tag="zp")
            for _ in range(NWARM):
                nc.tensor.matmul(
                    zp[:], lhsT=zw[:], rhs=zx[:], start=True, stop=True,
                    skip_group_check=True,
                )

            # ---- weights / constants ----
            # ALL large loads go on ONE queue (sync) in exact consumption
            # order: the 16 SDMA engines round-robin between active queues,
            # so a second queue halves the first's bandwidth (v5 trace: W0
            # on the scalar queue straggled to ~31us and stalled L0).
            # biasP first: the first ELU needs it, and ELUs recycle PSUM.
            biasP = wpool.tile([P, 3 * MH], F32, name="biasP", tag="biasP")
            nc.sync.dma_start(out=biasP[:], in_=bias_h[:])
            # W0 as 8 separate 512KB piece-tiles (2 k-tiles each) so chunk-0
            # matmuls depend only on the piece they read, not the whole 4MB.
            w0p = [
                wpool.tile([P, 2 * H], BF16, name=f"w0p{q}", tag=f"w0p{q}")
                for q in range(KD // 2)
            ]

            def load_w0_piece(q):
                nc.sync.dma_start(out=w0p[q][:], in_=w0_h[q * 256 : (q + 1) * 256, :])

            def w0slice(k, m):
                return w0p[k // 2][:, (k % 2) * H + m * P : (k % 2) * H + (m + 1) * P]

            w1all = wpool.tile([P, KH * H], BF16, name="w1all", tag="w1all")
            w2all = wpool.tile([P, KH * H], BF16, name="w2all", tag="w2all")
            wall = (None, w1all, w2all)

            def load_w12():
                nc.sync.dma_start(out=w1all[:], in_=w1_h[:])
                nc.sync.dma_start(out=w2all[:], in_=w2_h[:])

            wosb = wpool.tile([P, KH], BF16, name="wosb", tag="wosb")
            bosb = wpool.tile([1, 1], F32, name="bosb", tag="bosb")
            gaesb = gaepool.tile([P, 6 * T], F32, name="gaesb", tag="gaesb")
            rewsb = gaesb[:, 0 : 2 * T]
            discsb = gaesb[:, 2 * T : 4 * T]
            dlsb = gaesb[:, 4 * T : 6 * T]

            def load_consts():
                # tiny; parallel on the otherwise-idle scalar HWDGE queue
                nc.scalar.dma_start(out=wosb[:], in_=wo_h[:])
                nc.scalar.dma_start(out=bosb[:], in_=bo_h[:])
                nc.scalar.dma_start(out=gaesb[:], in_=gae_h[:])

            value_row = gaepool.tile([1, TOT], F32, name="value_row", tag="value_row")
            valP = gaepool.tile([P, 2 * SEG], F32, name="valPsb", tag="valPsb")
            dtt = gaepool.tile([P, 2 * T], F32, name="dtt", tag="dtt")
            adv = gaepool.tile([P, 2 * T], F32, name="adv", tag="adv")
            retP = gaepool.tile([P, 2 * T], F32, name="retP", tag="retP")

            ALUc = ALU

            def gae_half(p0, p1):
                # GAE for partitions [p0, p1): value_row cols [p0*34, p1*34).
                pp = slice(p0, p1)
                nc.sync.dma_start(
                    out=valP[pp, :], in_=value_row[0:1, p0 * 2 * SEG : p1 * 2 * SEG]
                )
                for s in range(2):
                    ss = slice(s * T, (s + 1) * T)
                    vnext = valP[pp, s * SEG : s * SEG + T]
                    vcur = valP[pp, s * SEG + 1 : s * SEG + 1 + T]
                    nc.vector.tensor_mul(dtt[pp, ss], discsb[pp, ss], vnext)
                    nc.vector.tensor_add(dtt[pp, ss], dtt[pp, ss], rewsb[pp, ss])
                    nc.vector.tensor_sub(dtt[pp, ss], dtt[pp, ss], vcur)
                    nc.vector.tensor_tensor_scan(
                        adv[pp, ss], dlsb[pp, ss], dtt[pp, ss], 0.0, ALUc.mult, ALUc.add
                    )
                    nc.vector.tensor_add(retP[pp, ss], adv[pp, ss], vcur)
                    nc.sync.dma_start(out=val_h[pp, ss], in_=vcur)
                nc.sync.dma_start(out=ret_h[pp, :], in_=retP[pp, :])

            # ---- streamed MLP over column chunks ----
            def elu(psm, li, m, hout, n):
                # ELU(z+b) = min(exp(z+b)-1, relu(z+b))
                bcol = biasP[:, li * MH + m : li * MH + m + 1]
                e = tmppool.tile([P, n], F32, name="e", tag="e")
                nc.scalar.activation(e[:], psm[:], ACTF.Exp, bias=bcol)
                rl = tmppool.tile([P, n], F32, name="rl", tag="rl")
                nc.vector.tensor_scalar(rl[:], psm[:], bcol, 0.0, ALU.add, ALU.max)
                nc.vector.scalar_tensor_tensor(
                    hout[:, m * n : (m + 1) * n], e[:], 1.0, rl[:],
                    ALU.subtract, ALU.min,
                )

            c0 = 0
            row0 = 0
            for ci, n in enumerate(CHUNKS):
                nrows = KD * P * n // 1024  # 1024 (n=512) or 512 (n=256)
                if ci == 0:
                    # chunk 0: four separate quarter-tiles (4 k-tiles each),
                    # each interleaved with the matching W0 pieces so the
                    # single DMA queue delivers in exact consumption order.
                    stq = []
                    for qi in range(4):
                        sq = stpool.tile([P, 4 * n], BF16, name=f"st0q{qi}", tag=f"st0q{qi}")
                        hr = nrows // 4
                        nc.sync.dma_start(
                            out=sq[:],
                            in_=statesT_h[row0 + qi * hr : row0 + (qi + 1) * hr, :],
                        )
                        stq.append(sq)
                        load_w0_piece(2 * qi)
                        load_w0_piece(2 * qi + 1)
                    load_w12()
                    load_consts()

                    def st0slice(k):
                        return stq[k // 4][:, (k % 4) * n : (k % 4 + 1) * n]

                else:
                    st_all = stpool.tile([P, KD * n], BF16, name="st", tag="st", bufs=2)
                    nc.sync.dma_start(
                        out=st_all[:], in_=statesT_h[row0 : row0 + nrows, :]
                    )
                row0 += nrows

                hs = []
                for li, nk in ((0, KD), (1, KH), (2, KH)):
                    rhs_src = hs[-1] if li else None
                    hout = hpool.tile([P, MH * n], BF16, name=f"h{li}", tag=f"h{li}", bufs=2)
                    if li == 0 and ci == 0:
                        # pass A: k-outer for m 0..4, consuming DMA pieces
                        # as they arrive (PE never waits for the full 6MB).
                        # 5 m-tiles: consumption 4.3us/k-group >= delivery
                        # 3.9us/group, so the PE never outruns the DMA.
                        psms = [
                            psApool.tile([P, n], F32, name="psm", tag="psm")
                            for _ in range(5)
                        ]
                        for k in range(KD):
                            for mi, psm in enumerate(psms):
                                nc.tensor.matmul(
                                    psm[:], lhsT=w0slice(k, mi), rhs=st0slice(k),
                                    start=(k == 0), stop=(k == KD - 1),
                                    skip_group_check=True,
                                )
                        for mi, psm in enumerate(psms):
                            elu(psm, 0, mi, hout, n)
                        # pass B: m-outer for m 5..7 (everything resident now)
                        for m in range(5, MH):
                            psm = psApool.tile([P, n], F32, name="psm", tag="psm")
                            for k in range(KD):
                                nc.tensor.matmul(
                                    psm[:], lhsT=w0slice(k, m), rhs=st0slice(k),
                                    start=(k == 0), stop=(k == KD - 1),
                                    skip_group_check=True,
                                )
                            elu(psm, 0, m, hout, n)
                        hs.append(hout)
                        continue
                    for m in range(MH):
                        psm = psApool.tile([P, n], F32, name="psm", tag="psm")
                        for k in range(nk):
                            if li == 0:
                                lhsT = w0slice(k, m)
                                rhs = st_all[:, k * n : (k + 1) * n]
                            else:
                                lhsT = wall[li][:, k * H + m * P : k * H + (m + 1) * P]
                                rhs = rhs_src[:, k * n : (k + 1) * n]
                            nc.tensor.matmul(
                                psm[:], lhsT=lhsT, rhs=rhs,
                                start=(k == 0), stop=(k == nk - 1),
                                skip_group_check=True,
                            )
                        elu(psm, li, m, hout, n)
                    hs.append(hout)

                # value head: Wo stationary [128,1] -> value lands [1, n]
                pv = psVpool.tile([1, n], F32, name="pv", tag="pv")
                for k in range(KH):
                    nc.tensor.matmul(
                        pv[:],
                        lhsT=wosb[:, k : k + 1],
                        rhs=hs[2][:, k * n : (k + 1) * n],
                        start=(k == 0),
                        stop=(k == KH - 1),
                        skip_group_check=True,
                    )
                nc.vector.tensor_scalar_add(
                    value_row[0:1, c0 : c0 + n], pv[:], bosb[0:1, 0:1]
                )
                c0 += n
                # GAE for partitions 0..63 (value_row cols < 2176) can run
                # as soon as chunk 4 (c0 = 2560) is done; it hides under
                # chunks 5-8. The rest runs in the tail.
                if ci == 4:
                    gae_half(0, 64)
            gae_half(64, P)

    nc.compile()
    return nc


def _get_nc():
    global _NC_CACHE
    if _NC_CACHE is None:
        _NC_CACHE = _build()
    return _NC_CACHE


def _pack_pmajor(w, nk):
    # [nk*128, cols] -> p-major [128, nk, cols] flattened back to same shape
    cols = w.shape[1]
    return np.ascontiguousarray(
        w.reshape(nk, P, cols).transpose(1, 0, 2).reshape(nk * P, cols)
    )


def _make_in_maps(inputs):
    import ml_dtypes

    BF = ml_dtypes.bfloat16
    states = np.asarray(inputs["states"], dtype=np.float32)
    reward = np.asarray(inputs["reward"], dtype=np.float32)
    cont = np.asarray(inputs["cont"], dtype=np.float32)

    # Feature-major states, b-major columns with reversed time:
    # full[d, b, r] = states[16-r, b, d] in bf16.
    st_bf = states.astype(BF)
    full = np.ascontiguousarray(st_bf[::-1].transpose(2, 1, 0))  # [D, B, TP1]

    W0 = np.asarray(inputs["W0"], np.float32).astype(BF)
    W1 = np.asarray(inputs["W1"], np.float32).astype(BF)
    W2 = np.asarray(inputs["W2"], np.float32).astype(BF)
    # W0: [8 parts, 128, 2, 1024] part-major then p-major
    W0t = np.ascontiguousarray(
        W0.reshape(8, 2, P, H).transpose(0, 2, 1, 3).reshape(D, H)
    )
    W1t = _pack_pmajor(W1, KH)
    W2t = _pack_pmajor(W2, KH)
    WoP = np.ascontiguousarray(
        np.asarray(inputs["Wo"], np.float32).astype(BF).reshape(KH, P).T
    )
    b3 = np.stack(
        [np.asarray(inputs[k], np.float32) for k in ("b0", "b1", "b2")]
    )  # [3, 1024]
    biasP = np.ascontiguousarray(b3.reshape(3, MH, P).transpose(2, 0, 1).reshape(P, 3 * MH))
    bo = np.ascontiguousarray(np.asarray(inputs["bo"], np.float32).reshape(1, 1))

    in_maps = []
    for c in range(NCORES):
        sl = slice(c * BC, (c + 1) * BC)
        # statesT for this core: [D, 4352] b-major/rev-t columns, then
        # per chunk: halves x [128, 8|16, n] p-major, flattened.
        stT = full[:, sl, :].reshape(D, TOT)
        blocks = []
        c0 = 0
        for ci, n in enumerate(CHUNKS):
            blk = stT[:, c0 : c0 + n].reshape(KD, P, n)  # [k, p, n]
            ndma = 4 if ci == 0 else 1
            kk = KD // ndma
            blocks.append(
                np.ascontiguousarray(
                    blk.reshape(ndma, kk, P, n).transpose(0, 2, 1, 3)
                ).reshape(-1)
            )
            c0 += n
        statesT = np.concatenate(blocks).reshape(D * TOT // 1024, 1024)

        # rewP[p, s*16+j] = reward[15-j, 2p+s]; disc uses cont[16-j].
        rr = reward[::-1, sl]  # [T, BC], rr[j] = reward[15-j]
        cc = cont[1:][::-1, sl]  # [T, BC], cc[j] = cont[16-j]
        rewP = rr.T.reshape(P, 2 * T)
        discP = (DISCOUNT * cc).T.reshape(P, 2 * T)
        dlP = (DISCOUNT * LAMBDA * cc).T.reshape(P, 2 * T)
        gaeP = np.ascontiguousarray(np.concatenate([rewP, discP, dlP], axis=1))
        in_maps.append(
            {
                "statesT": statesT,
                "W0t": W0t,
                "W1t": W1t,
                "W2t": W2t,
                "WoP": WoP,
                "biasP": biasP,
                "bo": bo,
                "gaeP": gaeP,
            }
        )
    return in_maps


def _run(inputs, trace=False):
    from concourse.bass_utils import run_bass_kernel_spmd

    nc = _get_nc()
    in_maps = _make_in_maps(inputs)
    bkr = run_bass_kernel_spmd(nc, in_maps, list(range(NCORES)), trace=trace)
    ret = np.empty((T, B), np.float32)
    val = np.empty((T, B), np.float32)
    for c in range(NCORES):
        sl = slice(c * BC, (c + 1) * BC)
        # retP[p, s*16+j] -> ret[15-j, 2p+s]
        rp = bkr.results[c]["retP"].reshape(P, 2, T)[:, :, ::-1]  # [p, s, t]
        vp = bkr.results[c]["valP"].reshape(P, 2, T)[:, :, ::-1]
        ret[:, sl] = rp.transpose(2, 0, 1).reshape(T, BC)
        val[:, sl] = vp.transpose(2, 0, 1).reshape(T, BC)
    return (ret, val), bkr


def kernel(**inputs):
    out, _ = _run(inputs, trace=False)
    return out


# revision 22
# speedup vs baseline: 1.0330x; 1.0081x over previous
"""Trainium2 Bass kernel for nn_Critic (MLP value function + GAE).

Sharding: batch B=2048 split across 8 NeuronCores (256 each). MLP params
replicated. The time recurrence (reverse GAE scan) is independent per batch
element, so no cross-core communication.

v3 strategy:
  - Single-pass bf16 matmuls everywhere (fp32 PSUM accumulate). Measured
    numpy emulation gives rel err ~5e-3 vs the 2e-2 gate.
  - states are transposed to feature-major bf16 on the HOST, so the kernel
    does zero PE transposes and zero hi/lo splits.
  - Column order is b-major with reversed time per batch segment:
    col = b*17 + r, r = 16-t. The MLP is row-independent so any column
    permutation works; this one makes the GAE a per-partition scan.
  - Work is streamed in chunks of N=512 columns (8x512 + 1x256): matmul
    free dim 512 = one PSUM bank, near-peak PE streaming.
  - All inputs are host-packed p-major so every SBUF tile loads with ONE
    large DMA (the v2 trace showed 164 small DMAs cost ~600ns of issue
    time each and stalled the PE for ~35us at start). The first chunk's
    states + W0 are split into 1MB pieces across both HWDGE queues so the
    PE can start after ~3us.
  - A few warm-up matmuls on zeroed tiles run during the initial DMA wait
    so the PE HAM clock-gate is at 2.4GHz when real work lands (v2 paid
    ~21us of cold-clock matmuls).
  - value head: Wo is the stationary operand ([128,1] slices) so values
    land in PSUM [1, N]; rows are appended into value_row [1, 4352] and
    reshaped once via a SBUF->SBUF DMA into valP [128, 34] (2 batch
    segments of 17 per partition).
  - GAE: a handful of [128,16] VectorE ops + tensor_tensor_scan per
    segment half. disc/dl (elementwise scalings of `cont`) and all
    reversals/permutations are host-side input prep.
"""

import sys

sys.path.insert(0, "/opt/trn_rl_repo")

import numpy as np

T, B, D, H = 16, 2048, 2048, 1024
NCORES = 8
BC = B // NCORES  # 256 batch per core
TP1 = T + 1
TOT = TP1 * BC  # 4352 MLP rows per core
DISCOUNT, LAMBDA = 0.99, 0.95
P = 128
KD = D // P  # 16 k-tiles for layer 0
KH = H // P  # 8 k-tiles for layers 1,2,out
MH = H // P  # 8 m-tiles of hidden units
CHUNKS = [512] * 8 + [256]  # sum = 4352
SEG = TP1  # 17 values per batch segment
NWARM = 12  # warm-up matmuls

_NC_CACHE = None


def _build():
    import concourse.bacc as bacc
    import concourse.mybir as mybir
    from concourse.tile import TileContext

    F32 = mybir.dt.float32
    BF16 = mybir.dt.bfloat16
    ALU = mybir.AluOpType
    ACTF = mybir.ActivationFunctionType

    nc = bacc.Bacc(None, target_bir_lowering=False, debug=False)

    # statesT: per chunk j (n cols), p-major halves: [2, 128, 8, n];
    # last chunk (n=256) is one [128, 16, n] block. Flattened to rows of 1024.
    statesT_h = nc.declare_dram_parameter("statesT", [D * TOT // 1024, 1024], BF16, isOutput=False)
    # W0: [4 parts, 128, 4, 1024] part/p-major; W1/W2: [128, 8, 1024] p-major.
    w0_h = nc.declare_dram_parameter("W0t", [D, H], BF16, isOutput=False)
    w1_h = nc.declare_dram_parameter("W1t", [H, H], BF16, isOutput=False)
    w2_h = nc.declare_dram_parameter("W2t", [H, H], BF16, isOutput=False)
    wo_h = nc.declare_dram_parameter("WoP", [P, KH], BF16, isOutput=False)
    wof_h = nc.declare_dram_parameter("WoPf", [P, KH], F32, isOutput=False)
    bias_h = nc.declare_dram_parameter("biasP", [P, 3 * MH], F32, isOutput=False)
    bo_h = nc.declare_dram_parameter("bo", [1, 1], F32, isOutput=False)
    gae_h = nc.declare_dram_parameter("gaeP", [P, 6 * T], F32, isOutput=False)
    ret_h = nc.declare_dram_parameter("retP", [P, 2 * T], F32, isOutput=True)
    val_h = nc.declare_dram_parameter("valP", [P, 2 * T], F32, isOutput=True)

    with TileContext(nc) as tc:
        with (
            tc.tile_pool(name="wpool", bufs=1) as wpool,
            tc.tile_pool(name="stpool", bufs=1) as stpool,
            tc.tile_pool(name="hpool", bufs=1) as hpool,
            tc.tile_pool(name="tmp", bufs=3) as tmppool,
            tc.tile_pool(name="accp", bufs=2) as accpool,
            tc.tile_pool(name="gae", bufs=1) as gaepool,
            tc.tile_pool(name="psA", bufs=5, space="PSUM") as psApool,
            tc.tile_pool(name="psV", bufs=2, space="PSUM") as psVpool,
            tc.tile_pool(name="psW", bufs=1, space="PSUM") as psWpool,
        ):
            # ---- PE warm-up on zeroed tiles (overlaps the first DMAs) ----
            zw = wpool.tile([P, P], BF16, name="zw", tag="zw")
            nc.vector.memset(zw[:], 0.0)
            zx = wpool.tile([P, 512], BF16, name="zx", tag="zx")
            nc.vector.memset(zx[:], 0.0)
            zp = psWpool.tile([P, 512], F32, name="zp", tag="zp")
            for _ in range(NWARM):
                nc.tensor.matmul(
                    zp[:], lhsT=zw[:], rhs=zx[:], start=True, stop=True,
                    skip_group_check=True,
                )

            # ---- weights / constants ----
            # ALL large loads go on ONE queue (sync) in exact consumption
            # order: the 16 SDMA engines round-robin between active queues,
            # so a second queue halves the first's bandwidth (v5 trace: W0
            # on the scalar queue straggled to ~31us and stalled L0).
            # biasP first: the first ELU needs it, and ELUs recycle PSUM.
            biasP = wpool.tile([P, 3 * MH], F32, name="biasP", tag="biasP")
            nc.sync.dma_start(out=biasP[:], in_=bias_h[:])
            # W0 as 8 separate 512KB piece-tiles (2 k-tiles each) so chunk-0
            # matmuls depend only on the piece they read, not the whole 4MB.
            w0p = [
                wpool.tile([P, 2 * H], BF16, name=f"w0p{q}", tag=f"w0p{q}")
                for q in range(KD // 2)
            ]

            def load_w0_piece(q):
                nc.sync.dma_start(out=w0p[q][:], in_=w0_h[q * 256 : (q + 1) * 256, :])

            def w0slice(k, m):
                return w0p[k // 2][:, (k % 2) * H + m * P : (k % 2) * H + (m + 1) * P]

            w1all = wpool.tile([P, KH * H], BF16, name="w1all", tag="w1all")
            w2all = wpool.tile([P, KH * H], BF16, name="w2all", tag="w2all")
            wall = (None, w1all, w2all)

            def load_w12():
                nc.sync.dma_start(out=w1all[:], in_=w1_h[:])
                nc.sync.dma_start(out=w2all[:], in_=w2_h[:])

            wosb = wpool.tile([P, KH], BF16, name="wosb", tag="wosb")
            wosbf = wpool.tile([P, KH], F32, name="wosbf", tag="wosbf")
            bosb = wpool.tile([1, 1], F32, name="bosb", tag="bosb")
            ones_sb = wpool.tile([P, 1], F32, name="ones_sb", tag="ones_sb")
            nc.vector.memset(ones_sb[:], 1.0)
            gaesb = gaepool.tile([P, 6 * T], F32, name="gaesb", tag="gaesb")
            rewsb = gaesb[:, 0 : 2 * T]
            discsb = gaesb[:, 2 * T : 4 * T]
            dlsb = gaesb[:, 4 * T : 6 * T]

            def load_consts():
                # tiny; parallel on the otherwise-idle scalar HWDGE queue
                nc.scalar.dma_start(out=wosb[:], in_=wo_h[:])
                nc.scalar.dma_start(out=wosbf[:], in_=wof_h[:])
                nc.scalar.dma_start(out=bosb[:], in_=bo_h[:])
                nc.scalar.dma_start(out=gaesb[:], in_=gae_h[:])

            value_row = gaepool.tile([1, TOT], F32, name="value_row", tag="value_row")
            valP = gaepool.tile([P, 2 * SEG], F32, name="valPsb", tag="valPsb")
            dtt = gaepool.tile([P, 2 * T], F32, name="dtt", tag="dtt")
            adv = gaepool.tile([P, 2 * T], F32, name="adv", tag="adv")
            retP = gaepool.tile([P, 2 * T], F32, name="retP", tag="retP")

            ALUc = ALU

            def gae_half(p0, p1):
                # GAE for partitions [p0, p1): value_row cols [p0*34, p1*34).
                pp = slice(p0, p1)
                nc.sync.dma_start(
                    out=valP[pp, :], in_=value_row[0:1, p0 * 2 * SEG : p1 * 2 * SEG]
                )
                for s in range(2):
                    ss = slice(s * T, (s + 1) * T)
                    vnext = valP[pp, s * SEG : s * SEG + T]
                    vcur = valP[pp, s * SEG + 1 : s * SEG + 1 + T]
                    nc.vector.tensor_mul(dtt[pp, ss], discsb[pp, ss], vnext)
                    nc.vector.tensor_add(dtt[pp, ss], dtt[pp, ss], rewsb[pp, ss])
                    nc.vector.tensor_sub(dtt[pp, ss], dtt[pp, ss], vcur)
                    nc.vector.tensor_tensor_scan(
                        adv[pp, ss], dlsb[pp, ss], dtt[pp, ss], 0.0, ALUc.mult, ALUc.add
                    )
                    nc.vector.tensor_add(retP[pp, ss], adv[pp, ss], vcur)
                    nc.sync.dma_start(out=val_h[pp, ss], in_=vcur)
                nc.sync.dma_start(out=ret_h[pp, :], in_=retP[pp, :])

            # ---- streamed MLP over column chunks ----
            # Deferred value head: for chunks 0..7 the per-chunk h3.Wo
            # contraction runs on the (slack) VectorE as 8 in-place
            # multiply-accumulates; the 128-partition reduction is ONE fp32
            # ones-vector matmul, emitted mid-way through the NEXT chunk's
            # L0 so the DVE chain latency never stalls the PE. This replaces
            # 8 bf16 N=n matmuls per chunk (saves ~0.85us/chunk of PE).
            pending = []

            def flush_head():
                if not pending:
                    return
                acc, pc0, pn = pending.pop()
                pv = psVpool.tile([1, pn], F32, name="pv", tag="pv")
                nc.tensor.matmul(
                    pv[:], lhsT=ones_sb[:], rhs=acc[:],
                    start=True, stop=True, skip_group_check=True,
                )
                nc.vector.tensor_scalar_add(
                    value_row[0:1, pc0 : pc0 + pn], pv[:], bosb[0:1, 0:1]
                )

            def elu(psm, li, m, hout, n):
                # ELU(z+b) = min(exp(z+b)-1, relu(z+b))
                bcol = biasP[:, li * MH + m : li * MH + m + 1]
                e = tmppool.tile([P, n], F32, name="e", tag="e")
                nc.scalar.activation(e[:], psm[:], ACTF.Exp, bias=bcol)
                rl = tmppool.tile([P, n], F32, name="rl", tag="rl")
                nc.vector.tensor_scalar(rl[:], psm[:], bcol, 0.0, ALU.add, ALU.max)
                nc.vector.scalar_tensor_tensor(
                    hout[:, m * n : (m + 1) * n], e[:], 1.0, rl[:],
                    ALU.subtract, ALU.min,
                )

            c0 = 0
            row0 = 0
            for ci, n in enumerate(CHUNKS):
                nrows = KD * P * n // 1024  # 1024 (n=512) or 512 (n=256)
                if ci == 0:
                    # chunk 0: four separate quarter-tiles (4 k-tiles each),
                    # each interleaved with the matching W0 pieces so the
                    # single DMA queue delivers in exact consumption order.
                    stq = []
                    for qi in range(4):
                        sq = stpool.tile([P, 4 * n], BF16, name=f"st0q{qi}", tag=f"st0q{qi}")
                        hr = nrows // 4
                        nc.sync.dma_start(
                            out=sq[:],
                            in_=statesT_h[row0 + qi * hr : row0 + (qi + 1) * hr, :],
                        )
                        stq.append(sq)
                        load_w0_piece(2 * qi)
                        load_w0_piece(2 * qi + 1)
                    load_w12()
                    load_consts()

                    def st0slice(k):
                        return stq[k // 4][:, (k % 4) * n : (k % 4 + 1) * n]

                else:
                    st_all = stpool.tile([P, KD * n], BF16, name="st", tag="st", bufs=2)
                    nc.sync.dma_start(
                        out=st_all[:], in_=statesT_h[row0 : row0 + nrows, :]
                    )
                row0 += nrows

                hs = []
                for li, nk in ((0, KD), (1, KH), (2, KH)):
                    rhs_src = hs[-1] if li else None
                    hout = hpool.tile([P, MH * n], BF16, name=f"h{li}", tag=f"h{li}", bufs=2)
                    if li == 0 and ci == 0:
                        # pass A: k-outer for m 0..4, consuming DMA pieces
                        # as they arrive (PE never waits for the full 6MB).
                        # 5 m-tiles: consumption 4.3us/k-group >= delivery
                        # 3.9us/group, so the PE never outruns the DMA.
                        psms = [
                            psApool.tile([P, n], F32, name="psm", tag="psm")
                            for _ in range(5)
                        ]
                        for k in range(KD):
                            for mi, psm in enumerate(psms):
                                nc.tensor.matmul(
                                    psm[:], lhsT=w0slice(k, mi), rhs=st0slice(k),
                                    start=(k == 0), stop=(k == KD - 1),
                                    skip_group_check=True,
                                )
                        for mi, psm in enumerate(psms):
                            elu(psm, 0, mi, hout, n)
                        # pass B: m-outer for m 5..7 (everything resident now)
                        for m in range(5, MH):
                            psm = psApool.tile([P, n], F32, name="psm", tag="psm")
                            for k in range(KD):
                                nc.tensor.matmul(
                                    psm[:], lhsT=w0slice(k, m), rhs=st0slice(k),
                                    start=(k == 0), stop=(k == KD - 1),
                                    skip_group_check=True,
                                )
                            elu(psm, 0, m, hout, n)
                        hs.append(hout)
                        continue
                    for m in range(MH):
                        psm = psApool.tile([P, n], F32, name="psm", tag="psm")
                        for k in range(nk):
                            if li == 0:
                                lhsT = w0slice(k, m)
                                rhs = st_all[:, k * n : (k + 1) * n]
                            else:
                                lhsT = wall[li][:, k * H + m * P : k * H + (m + 1) * P]
                                rhs = rhs_src[:, k * n : (k + 1) * n]
                            nc.tensor.matmul(
                                psm[:], lhsT=lhsT, rhs=rhs,
                                start=(k == 0), stop=(k == nk - 1),
                                skip_group_check=True,
                            )
                        elu(psm, li, m, hout, n)
                        if li == 0 and m == 2:
                            flush_head()
                    hs.append(hout)

                if ci < len(CHUNKS) - 1:
                    # value head via DVE: acc = sum_k h3_k * wo_k (fp32)
                    acc = accpool.tile([P, n], F32, name="acc", tag="acc")
                    nc.vector.tensor_scalar_mul(acc[:], hs[2][:, 0:n], wosbf[:, 0:1])
                    for k in range(1, KH):
                        nc.vector.scalar_tensor_tensor(
                            acc[:], hs[2][:, k * n : (k + 1) * n], wosbf[:, k : k + 1],
                            acc[:], ALU.mult, ALU.add,
                        )
                    pending.append((acc, c0, n))
                else:
                    # last chunk: direct PE head (keeps the tail short)
                    pv = psVpool.tile([1, n], F32, name="pv", tag="pv")
                    for k in range(KH):
                        nc.tensor.matmul(
                            pv[:],
                            lhsT=wosb[:, k : k + 1],
                            rhs=hs[2][:, k * n : (k + 1) * n],
                            start=(k == 0),
                            stop=(k == KH - 1),
                            skip_group_check=True,
                        )
                    nc.vector.tensor_scalar_add(
                        value_row[0:1, c0 : c0 + n], pv[:], bosb[0:1, 0:1]
                    )
                c0 += n
                # GAE for partitions 0..63 (value_row cols < 2176) can run
                # once chunks 0..4's heads are written (head(4) flushes
                # during chunk 5's L0); it hides under chunks 6-8.
                if ci == 5:
                    gae_half(0, 64)
            gae_half(64, P)

    nc.compile()
    return nc


def _get_nc():
    global _NC_CACHE
    if _NC_CACHE is None:
        _NC_CACHE = _build()
    return _NC_CACHE


def _pack_pmajor(w, nk):
    # [nk*128, cols] -> p-major [128, nk, cols] flattened back to same shape
    cols = w.shape[1]
    return np.ascontiguousarray(
        w.reshape(nk, P, cols).transpose(1, 0, 2).reshape(nk * P, cols)
    )


def _make_in_maps(inputs):
    import ml_dtypes

    BF = ml_dtypes.bfloat16
    states = np.asarray(inputs["states"], dtype=np.float32)
    reward = np.asarray(inputs["reward"], dtype=np.float32)
    cont = np.asarray(inputs["cont"], dtype=np.float32)

    # Feature-major states, b-major columns with reversed time:
    # full[d, b, r] = states[16-r, b, d] in bf16.
    st_bf = states.astype(BF)
    full = np.ascontiguousarray(st_bf[::-1].transpose(2, 1, 0))  # [D, B, TP1]

    W0 = np.asarray(inputs["W0"], np.float32).astype(BF)
    W1 = np.asarray(inputs["W1"], np.float32).astype(BF)
    W2 = np.asarray(inputs["W2"], np.float32).astype(BF)
    # W0: [8 parts, 128, 2, 1024] part-major then p-major
    W0t = np.ascontiguousarray(
        W0.reshape(8, 2, P, H).transpose(0, 2, 1, 3).reshape(D, H)
    )
    W1t = _pack_pmajor(W1, KH)
    W2t = _pack_pmajor(W2, KH)
    WoP = np.ascontiguousarray(
        np.asarray(inputs["Wo"], np.float32).astype(BF).reshape(KH, P).T
    )
    WoPf = np.ascontiguousarray(WoP.astype(np.float32))
    b3 = np.stack(
        [np.asarray(inputs[k], np.float32) for k in ("b0", "b1", "b2")]
    )  # [3, 1024]
    biasP = np.ascontiguousarray(b3.reshape(3, MH, P).transpose(2, 0, 1).reshape(P, 3 * MH))
    bo = np.ascontiguousarray(np.asarray(inputs["bo"], np.float32).reshape(1, 1))

    in_maps = []
    for c in range(NCORES):
        sl = slice(c * BC, (c + 1) * BC)
        # statesT for this core: [D, 4352] b-major/rev-t columns, then
        # per chunk: halves x [128, 8|16, n] p-major, flattened.
        stT = full[:, sl, :].reshape(D, TOT)
        blocks = []
        c0 = 0
        for ci, n in enumerate(CHUNKS):
            blk = stT[:, c0 : c0 + n].reshape(KD, P, n)  # [k, p, n]
            ndma = 4 if ci == 0 else 1
            kk = KD // ndma
            blocks.append(
                np.ascontiguousarray(
                    blk.reshape(ndma, kk, P, n).transpose(0, 2, 1, 3)
                ).reshape(-1)
            )
            c0 += n
        statesT = np.concatenate(blocks).reshape(D * TOT // 1024, 1024)

        # rewP[p, s*16+j] = reward[15-j, 2p+s]; disc uses cont[16-j].
        rr = reward[::-1, sl]  # [T, BC], rr[j] = reward[15-j]
        cc = cont[1:][::-1, sl]  # [T, BC], cc[j] = cont[16-j]
        rewP = rr.T.reshape(P, 2 * T)
        discP = (DISCOUNT * cc).T.reshape(P, 2 * T)
        dlP = (DISCOUNT * LAMBDA * cc).T.reshape(P, 2 * T)
        gaeP = np.ascontiguousarray(np.concatenate([rewP, discP, dlP], axis=1))
        in_maps.append(
            {
                "statesT": statesT,
                "W0t": W0t,
                "W1t": W1t,
                "W2t": W2t,
                "WoP": WoP,
                "WoPf": WoPf,
                "biasP": biasP,
                "bo": bo,
                "gaeP": gaeP,
            }
        )
    return in_maps


def _run(inputs, trace=False):
    from concourse.bass_utils import run_bass_kernel_spmd

    nc = _get_nc()
    in_maps = _make_in_maps(inputs)
    bkr = run_bass_kernel_spmd(nc, in_maps, list(range(NCORES)), trace=trace)
    ret = np.empty((T, B), np.float32)
    val = np.empty((T, B), np.float32)
    for c in range(NCORES):
        sl = slice(c * BC, (c + 1) * BC)
        # retP[p, s*16+j] -> ret[15-j, 2p+s]
        rp = bkr.results[c]["retP"].reshape(P, 2, T)[:, :, ::-1]  # [p, s, t]
        vp = bkr.results[c]["valP"].reshape(P, 2, T)[:, :, ::-1]
        ret[:, sl] = rp.transpose(2, 0, 1).reshape(T, BC)
        val[:, sl] = vp.transpose(2, 0, 1).reshape(T, BC)
    return (ret, val), bkr


def kernel(**inputs):
    out, _ = _run(inputs, trace=False)
    return out


# revision 23
# speedup vs baseline: 1.0363x; 1.0031x over previous
"""Trainium2 Bass kernel for nn_Critic (MLP value function + GAE).

Sharding: batch B=2048 split across 8 NeuronCores (256 each). MLP params
replicated. The time recurrence (reverse GAE scan) is independent per batch
element, so no cross-core communication.

Strategy (measured ~505us vs 1841us baseline; MFU ~90%):
  - Single-pass bf16 matmuls everywhere (fp32 PSUM accumulate). Measured
    numpy emulation gives rel err ~5e-3 vs the 2e-2 gate (3x less matmul
    work than a bf16 hi/lo split).
  - states are transposed to feature-major bf16 on the HOST, so the kernel
    does zero PE transposes and zero hi/lo splits.
  - Column order is b-major with reversed time per batch segment:
    col = b*17 + r, r = 16-t. The MLP is row-independent so any column
    permutation works; this one makes the GAE a per-partition scan.
  - Work is streamed in chunks of N=512 columns (8x512 + 1x256): matmul
    free dim 512 = one PSUM bank, near-peak PE streaming (213ns/MM).
  - All inputs are host-packed p-major so SBUF tiles load with few large
    DMAs (each dma_start costs ~600ns issue + ~2us completion latency).
    ALL large loads share ONE HWDGE queue in exact consumption order (the
    16 SDMA engines round-robin across active queues, so a second queue
    halves the first's bandwidth).
  - Chunk 0 streams from 512KB piece-TILES (separate tiles => fine-grained
    deps) with layer 0 k-outer over m=0..4 so the PE consumes pieces as
    they land; 12 warm-up matmuls on zeroed tiles cover the ~7us engine
    boot and keep the PE HAM clock-gate at 2.4GHz when real work starts.
  - value head: for chunks 0..7, h3.Wo runs on the slack VectorE as 8
    in-place multiply-accumulates; the 128-partition reduction is one fp32
    ones-vector matmul deferred into the next chunk's L0 (saves ~0.85us
    PE per chunk vs 8 bf16 N=512 matmuls). Values land in value_row
    [1, 4352], reshaped once via SBUF->SBUF DMA into valP [128, 34].
  - GAE: a handful of [128,16] VectorE ops + tensor_tensor_scan per
    segment half, split into two partition halves so the first runs
    hidden under chunks 6-8. disc/dl (elementwise scalings of `cont`)
    and all reversals/permutations are host-side input prep.
"""

import sys

sys.path.insert(0, "/opt/trn_rl_repo")

import numpy as np

T, B, D, H = 16, 2048, 2048, 1024
NCORES = 8
BC = B // NCORES  # 256 batch per core
TP1 = T + 1
TOT = TP1 * BC  # 4352 MLP rows per core
DISCOUNT, LAMBDA = 0.99, 0.95
P = 128
KD = D // P  # 16 k-tiles for layer 0
KH = H // P  # 8 k-tiles for layers 1,2,out
MH = H // P  # 8 m-tiles of hidden units
CHUNKS = [512] * 8 + [256]  # sum = 4352
SEG = TP1  # 17 values per batch segment
NWARM = 12  # warm-up matmuls

_NC_CACHE = None


def _build():
    import concourse.bacc as bacc
    import concourse.mybir as mybir
    from concourse.tile import TileContext

    F32 = mybir.dt.float32
    BF16 = mybir.dt.bfloat16
    ALU = mybir.AluOpType
    ACTF = mybir.ActivationFunctionType

    nc = bacc.Bacc(None, target_bir_lowering=False, debug=False)

    # statesT: chunk 0 as 4 p-major quarters [4, 128, 4, n]; other chunks
    # one [128, 16, n] p-major block each. Flattened to rows of 1024.
    statesT_h = nc.declare_dram_parameter("statesT", [D * TOT // 1024, 1024], BF16, isOutput=False)
    # W0: [8 parts, 128, 2, 1024] part/p-major; W1/W2: [128, 8, 1024] p-major.
    w0_h = nc.declare_dram_parameter("W0t", [D, H], BF16, isOutput=False)
    w1_h = nc.declare_dram_parameter("W1t", [H, H], BF16, isOutput=False)
    w2_h = nc.declare_dram_parameter("W2t", [H, H], BF16, isOutput=False)
    wo_h = nc.declare_dram_parameter("WoP", [P, KH], BF16, isOutput=False)
    wof_h = nc.declare_dram_parameter("WoPf", [P, KH], F32, isOutput=False)
    bias_h = nc.declare_dram_parameter("biasP", [P, 3 * MH], F32, isOutput=False)
    bo_h = nc.declare_dram_parameter("bo", [1, 1], F32, isOutput=False)
    gae_h = nc.declare_dram_parameter("gaeP", [P, 6 * T], F32, isOutput=False)
    ret_h = nc.declare_dram_parameter("retP", [P, 2 * T], F32, isOutput=True)
    val_h = nc.declare_dram_parameter("valP", [P, 2 * T], F32, isOutput=True)

    with TileContext(nc) as tc:
        with (
            tc.tile_pool(name="wpool", bufs=1) as wpool,
            tc.tile_pool(name="stpool", bufs=1) as stpool,
            tc.tile_pool(name="hpool", bufs=1) as hpool,
            tc.tile_pool(name="tmp", bufs=3) as tmppool,
            tc.tile_pool(name="accp", bufs=2) as accpool,
            tc.tile_pool(name="gae", bufs=1) as gaepool,
            tc.tile_pool(name="psA", bufs=5, space="PSUM") as psApool,
            tc.tile_pool(name="psV", bufs=2, space="PSUM") as psVpool,
            tc.tile_pool(name="psW", bufs=1, space="PSUM") as psWpool,
        ):
            # ---- PE warm-up on zeroed tiles (overlaps the first DMAs) ----
            zw = wpool.tile([P, P], BF16, name="zw", tag="zw")
            nc.vector.memset(zw[:], 0.0)
            zx = wpool.tile([P, 512], BF16, name="zx", tag="zx")
            nc.vector.memset(zx[:], 0.0)
            zp = psWpool.tile([P, 512], F32, name="zp", tag="zp")
            for _ in range(NWARM):
                nc.tensor.matmul(
                    zp[:], lhsT=zw[:], rhs=zx[:], start=True, stop=True,
                    skip_group_check=True,
                )

            # ---- weights / constants ----
            # ALL large loads go on ONE queue (sync) in exact consumption
            # order: the 16 SDMA engines round-robin between active queues,
            # so a second queue halves the first's bandwidth (v5 trace: W0
            # on the scalar queue straggled to ~31us and stalled L0).
            # biasP first: the first ELU needs it, and ELUs recycle PSUM.
            biasP = wpool.tile([P, 3 * MH], F32, name="biasP", tag="biasP")
            nc.sync.dma_start(out=biasP[:], in_=bias_h[:])
            # W0 as 8 separate 512KB piece-tiles (2 k-tiles each) so chunk-0
            # matmuls depend only on the piece they read, not the whole 4MB.
            w0p = [
                wpool.tile([P, 2 * H], BF16, name=f"w0p{q}", tag=f"w0p{q}")
                for q in range(KD // 2)
            ]

            def load_w0_piece(q):
                nc.sync.dma_start(out=w0p[q][:], in_=w0_h[q * 256 : (q + 1) * 256, :])

            def w0slice(k, m):
                return w0p[k // 2][:, (k % 2) * H + m * P : (k % 2) * H + (m + 1) * P]

            w1all = wpool.tile([P, KH * H], BF16, name="w1all", tag="w1all")
            w2all = wpool.tile([P, KH * H], BF16, name="w2all", tag="w2all")
            wall = (None, w1all, w2all)

            def load_w12():
                nc.sync.dma_start(out=w1all[:], in_=w1_h[:])
                nc.sync.dma_start(out=w2all[:], in_=w2_h[:])

            wosb = wpool.tile([P, KH], BF16, name="wosb", tag="wosb")
            wosbf = wpool.tile([P, KH], F32, name="wosbf", tag="wosbf")
            bosb = wpool.tile([1, 1], F32, name="bosb", tag="bosb")
            ones_sb = wpool.tile([P, 1], F32, name="ones_sb", tag="ones_sb")
            nc.vector.memset(ones_sb[:], 1.0)
            gaesb = gaepool.tile([P, 6 * T], F32, name="gaesb", tag="gaesb")
            rewsb = gaesb[:, 0 : 2 * T]
            discsb = gaesb[:, 2 * T : 4 * T]
            dlsb = gaesb[:, 4 * T : 6 * T]

            def load_consts():
                # tiny; parallel on the otherwise-idle scalar HWDGE queue
                nc.scalar.dma_start(out=wosb[:], in_=wo_h[:])
                nc.scalar.dma_start(out=wosbf[:], in_=wof_h[:])
                nc.scalar.dma_start(out=bosb[:], in_=bo_h[:])
                nc.scalar.dma_start(out=gaesb[:], in_=gae_h[:])

            value_row = gaepool.tile([1, TOT], F32, name="value_row", tag="value_row")
            valP = gaepool.tile([P, 2 * SEG], F32, name="valPsb", tag="valPsb")
            dtt = gaepool.tile([P, 2 * T], F32, name="dtt", tag="dtt")
            adv = gaepool.tile([P, 2 * T], F32, name="adv", tag="adv")
            retP = gaepool.tile([P, 2 * T], F32, name="retP", tag="retP")

            ALUc = ALU

            def gae_half(p0, p1):
                # GAE for partitions [p0, p1): value_row cols [p0*34, p1*34).
                pp = slice(p0, p1)
                nc.sync.dma_start(
                    out=valP[pp, :], in_=value_row[0:1, p0 * 2 * SEG : p1 * 2 * SEG]
                )
                for s in range(2):
                    ss = slice(s * T, (s + 1) * T)
                    vnext = valP[pp, s * SEG : s * SEG + T]
                    vcur = valP[pp, s * SEG + 1 : s * SEG + 1 + T]
                    nc.vector.tensor_mul(dtt[pp, ss], discsb[pp, ss], vnext)
                    nc.vector.tensor_add(dtt[pp, ss], dtt[pp, ss], rewsb[pp, ss])
                    nc.vector.tensor_sub(dtt[pp, ss], dtt[pp, ss], vcur)
                    nc.vector.tensor_tensor_scan(
                        adv[pp, ss], dlsb[pp, ss], dtt[pp, ss], 0.0, ALUc.mult, ALUc.add
                    )
                    nc.vector.tensor_add(retP[pp, ss], adv[pp, ss], vcur)
                    nc.sync.dma_start(out=val_h[pp, ss], in_=vcur)
                nc.sync.dma_start(out=ret_h[pp, :], in_=retP[pp, :])

            # ---- streamed MLP over column chunks ----
            # Deferred value head: for chunks 0..7 the per-chunk h3.Wo
            # contraction runs on the (slack) VectorE as 8 in-place
            # multiply-accumulates; the 128-partition reduction is ONE fp32
            # ones-vector matmul, emitted mid-way through the NEXT chunk's
            # L0 so the DVE chain latency never stalls the PE. This replaces
            # 8 bf16 N=n matmuls per chunk (saves ~0.85us/chunk of PE).
            pending = []

            def flush_head():
                if not pending:
                    return
                acc, pc0, pn = pending.pop()
                pv = psVpool.tile([1, pn], F32, name="pv", tag="pv")
                nc.tensor.matmul(
                    pv[:], lhsT=ones_sb[:], rhs=acc[:],
                    start=True, stop=True, skip_group_check=True,
                )
                nc.vector.tensor_scalar_add(
                    value_row[0:1, pc0 : pc0 + pn], pv[:], bosb[0:1, 0:1]
                )

            def elu(psm, li, m, hout, n):
                # ELU(z+b) = min(exp(z+b)-1, relu(z+b))
                bcol = biasP[:, li * MH + m : li * MH + m + 1]
                e = tmppool.tile([P, n], F32, name="e", tag="e")
                nc.scalar.activation(e[:], psm[:], ACTF.Exp, bias=bcol)
                rl = tmppool.tile([P, n], F32, name="rl", tag="rl")
                nc.vector.tensor_scalar(rl[:], psm[:], bcol, 0.0, ALU.add, ALU.max)
                nc.vector.scalar_tensor_tensor(
                    hout[:, m * n : (m + 1) * n], e[:], 1.0, rl[:],
                    ALU.subtract, ALU.min,
                )

            c0 = 0
            row0 = 0
            for ci, n in enumerate(CHUNKS):
                nrows = KD * P * n // 1024  # 1024 (n=512) or 512 (n=256)
                if ci == 0:
                    # chunk 0: four separate quarter-tiles (4 k-tiles each),
                    # each interleaved with the matching W0 pieces so the
                    # single DMA queue delivers in exact consumption order.
                    stq = []
                    for qi in range(4):
                        sq = stpool.tile([P, 4 * n], BF16, name=f"st0q{qi}", tag=f"st0q{qi}")
                        hr = nrows // 4
                        nc.sync.dma_start(
                            out=sq[:],
                            in_=statesT_h[row0 + qi * hr : row0 + (qi + 1) * hr, :],
                        )
                        stq.append(sq)
                        load_w0_piece(2 * qi)
                        load_w0_piece(2 * qi + 1)
                    load_w12()
                    load_consts()

                    def st0slice(k):
                        return stq[k // 4][:, (k % 4) * n : (k % 4 + 1) * n]

                else:
                    st_all = stpool.tile([P, KD * n], BF16, name="st", tag="st", bufs=2)
                    nc.sync.dma_start(
                        out=st_all[:], in_=statesT_h[row0 : row0 + nrows, :]
                    )
                row0 += nrows

                hs = []
                for li, nk in ((0, KD), (1, KH), (2, KH)):
                    rhs_src = hs[-1] if li else None
                    hout = hpool.tile([P, MH * n], BF16, name=f"h{li}", tag=f"h{li}", bufs=2)
                    if li == 0 and ci == 0:
                        # pass A: k-outer for m 0..4, consuming DMA pieces
                        # as they arrive (PE never waits for the full 6MB).
                        # 5 m-tiles: consumption 4.3us/k-group >= delivery
                        # 3.9us/group, so the PE never outruns the DMA.
                        psms = [
                            psApool.tile([P, n], F32, name="psm", tag="psm")
                            for _ in range(5)
                        ]
                        for k in range(KD):
                            for mi, psm in enumerate(psms):
                                nc.tensor.matmul(
                                    psm[:], lhsT=w0slice(k, mi), rhs=st0slice(k),
                                    start=(k == 0), stop=(k == KD - 1),
                                    skip_group_check=True,
                                )
                        for mi, psm in enumerate(psms):
                            elu(psm, 0, mi, hout, n)
                        # pass B: m-outer for m 5..7 (everything resident now)
                        for m in range(5, MH):
                            psm = psApool.tile([P, n], F32, name="psm", tag="psm")
                            for k in range(KD):
                                nc.tensor.matmul(
                                    psm[:], lhsT=w0slice(k, m), rhs=st0slice(k),
                                    start=(k == 0), stop=(k == KD - 1),
                                    skip_group_check=True,
                                )
                            elu(psm, 0, m, hout, n)
                        hs.append(hout)
                        continue
                    for m in range(MH):
                        psm = psApool.tile([P, n], F32, name="psm", tag="psm")
                        for k in range(nk):
                            if li == 0:
                                lhsT = w0slice(k, m)
                                rhs = st_all[:, k * n : (k + 1) * n]
                            else:
                                lhsT = wall[li][:, k * H + m * P : k * H + (m + 1) * P]
                                rhs = rhs_src[:, k * n : (k + 1) * n]
                            nc.tensor.matmul(
                                psm[:], lhsT=lhsT, rhs=rhs,
                                start=(k == 0), stop=(k == nk - 1),
                                skip_group_check=True,
                            )
                        elu(psm, li, m, hout, n)
                        if li == 0 and m == 2:
                            flush_head()
                    hs.append(hout)

                if ci < len(CHUNKS) - 1:
                    # value head via DVE: acc = sum_k h3_k * wo_k (fp32)
                    acc = accpool.tile([P, n], F32, name="acc", tag="acc")
                    nc.vector.tensor_scalar_mul(acc[:], hs[2][:, 0:n], wosbf[:, 0:1])
                    for k in range(1, KH):
                        nc.vector.scalar_tensor_tensor(
                            acc[:], hs[2][:, k * n : (k + 1) * n], wosbf[:, k : k + 1],
                            acc[:], ALU.mult, ALU.add,
                        )
                    pending.append((acc, c0, n))
                else:
                    # last chunk: direct PE head (keeps the tail short)
                    pv = psVpool.tile([1, n], F32, name="pv", tag="pv")
                    for k in range(KH):
                        nc.tensor.matmul(
                            pv[:],
                            lhsT=wosb[:, k : k + 1],
                            rhs=hs[2][:, k * n : (k + 1) * n],
                            start=(k == 0),
                            stop=(k == KH - 1),
                            skip_group_check=True,
                        )
                    nc.vector.tensor_scalar_add(
                        value_row[0:1, c0 : c0 + n], pv[:], bosb[0:1, 0:1]
                    )
                c0 += n
                # GAE for partitions 0..63 (value_row cols < 2176) can run
                # once chunks 0..4's heads are written (head(4) flushes
                # during chunk 5's L0); it hides under chunks 6-8.
                if ci == 5:
                    gae_half(0, 64)
            gae_half(64, P)

    nc.compile()
    return nc


def _get_nc():
    global _NC_CACHE
    if _NC_CACHE is None:
        _NC_CACHE = _build()
    return _NC_CACHE


def _pack_pmajor(w, nk):
    # [nk*128, cols] -> p-major [128, nk, cols] flattened back to same shape
    cols = w.shape[1]
    return np.ascontiguousarray(
        w.reshape(nk, P, cols).transpose(1, 0, 2).reshape(nk * P, cols)
    )


def _make_in_maps(inputs):
    import ml_dtypes

    BF = ml_dtypes.bfloat16
    states = np.asarray(inputs["states"], dtype=np.float32)
    reward = np.asarray(inputs["reward"], dtype=np.float32)
    cont = np.asarray(inputs["cont"], dtype=np.float32)

    # Feature-major states, b-major columns with reversed time:
    # full[d, b, r] = states[16-r, b, d] in bf16.
    st_bf = states.astype(BF)
    full = np.ascontiguousarray(st_bf[::-1].transpose(2, 1, 0))  # [D, B, TP1]

    W0 = np.asarray(inputs["W0"], np.float32).astype(BF)
    W1 = np.asarray(inputs["W1"], np.float32).astype(BF)
    W2 = np.asarray(inputs["W2"], np.float32).astype(BF)
    # W0: [8 parts, 128, 2, 1024] part-major then p-major
    W0t = np.ascontiguousarray(
        W0.reshape(8, 2, P, H).transpose(0, 2, 1, 3).reshape(D, H)
    )
    W1t = _pack_pmajor(W1, KH)
    W2t = _pack_pmajor(W2, KH)
    WoP = np.ascontiguousarray(
        np.asarray(inputs["Wo"], np.float32).astype(BF).reshape(KH, P).T
    )
    WoPf = np.ascontiguousarray(WoP.astype(np.float32))
    b3 = np.stack(
        [np.asarray(inputs[k], np.float32) for k in ("b0", "b1", "b2")]
    )  # [3, 1024]
    biasP = np.ascontiguousarray(b3.reshape(3, MH, P).transpose(2, 0, 1).reshape(P, 3 * MH))
    bo = np.ascontiguousarray(np.asarray(inputs["bo"], np.float32).reshape(1, 1))

    in_maps = []
    for c in range(NCORES):
        sl = slice(c * BC, (c + 1) * BC)
        # statesT for this core: [D, 4352] b-major/rev-t columns, then
        # per chunk: halves x [128, 8|16, n] p-major, flattened.
        stT = full[:, sl, :].reshape(D, TOT)
        blocks = []
        c0 = 0
        for ci, n in enumerate(CHUNKS):
            blk = stT[:, c0 : c0 + n].reshape(KD, P, n)  # [k, p, n]
            ndma = 4 if ci == 0 else 1
            kk = KD // ndma
            blocks.append(
                np.ascontiguousarray(
                    blk.reshape(ndma, kk, P, n).transpose(0, 2, 1, 3)
                ).reshape(-1)
            )
            c0 += n
        statesT = np.concatenate(blocks).reshape(D * TOT // 1024, 1024)

        # rewP[p, s*16+j] = reward[15-j, 2p+s]; disc uses cont[16-j].
        rr = reward[::-1, sl]  # [T, BC], rr[j] = reward[15-j]
        cc = cont[1:][::-1, sl]  # [T, BC], cc[j] = cont[16-j]
        rewP = rr.T.reshape(P, 2 * T)
        discP = (DISCOUNT * cc).T.reshape(P, 2 * T)
        dlP = (DISCOUNT * LAMBDA * cc).T.reshape(P, 2 * T)
        gaeP = np.ascontiguousarray(np.concatenate([rewP, discP, dlP], axis=1))
        in_maps.append(
            {
                "statesT": statesT,
                "W0t": W0t,
                "W1t": W1t,
                "W2t": W2t,
                "WoP": WoP,
                "WoPf": WoPf,
                "biasP": biasP,
                "bo": bo,
                "gaeP": gaeP,
            }
        )
    return in_maps


def _run(inputs, trace=False):
    from concourse.bass_utils import run_bass_kernel_spmd

    nc = _get_nc()
    in_maps = _make_in_maps(inputs)
    bkr = run_bass_kernel_spmd(nc, in_maps, list(range(NCORES)), trace=trace)
    ret = np.empty((T, B), np.float32)
    val = np.empty((T, B), np.float32)
    for c in range(NCORES):
        sl = slice(c * BC, (c + 1) * BC)
        # retP[p, s*16+j] -> ret[15-j, 2p+s]
        rp = bkr.results[c]["retP"].reshape(P, 2, T)[:, :, ::-1]  # [p, s, t]
        vp = bkr.results[c]["valP"].reshape(P, 2, T)[:, :, ::-1]
        ret[:, sl] = rp.transpose(2, 0, 1).reshape(T, BC)
        val[:, sl] = vp.transpose(2, 0, 1).reshape(T, BC)
    return (ret, val), bkr


def kernel(**inputs):
    out, _ = _run(inputs, trace=False)
    return out


# revision 24
# speedup vs baseline: 1.0522x; 1.0154x over previous
"""Trainium2 Bass kernel for nn_Critic (MLP value function + GAE).

Sharding: batch B=2048 split across 8 NeuronCores (256 each). MLP params
replicated. The time recurrence (reverse GAE scan) is independent per batch
element, so no cross-core communication.

Strategy (measured ~505us vs 1841us baseline; MFU ~90%):
  - Single-pass bf16 matmuls everywhere (fp32 PSUM accumulate). Measured
    numpy emulation gives rel err ~5e-3 vs the 2e-2 gate (3x less matmul
    work than a bf16 hi/lo split).
  - states are transposed to feature-major bf16 on the HOST, so the kernel
    does zero PE transposes and zero hi/lo splits.
  - Column order is b-major with reversed time per batch segment:
    col = b*17 + r, r = 16-t. The MLP is row-independent so any column
    permutation works; this one makes the GAE a per-partition scan.
  - Work is streamed in chunks of N=512 columns (8x512 + 1x256): matmul
    free dim 512 = one PSUM bank, near-peak PE streaming (213ns/MM).
  - All inputs are host-packed p-major so SBUF tiles load with few large
    DMAs (each dma_start costs ~600ns issue + ~2us completion latency).
    ALL large loads share ONE HWDGE queue in exact consumption order (the
    16 SDMA engines round-robin across active queues, so a second queue
    halves the first's bandwidth).
  - Chunk 0 streams from 512KB piece-TILES (separate tiles => fine-grained
    deps) with layer 0 k-outer over m=0..4 so the PE consumes pieces as
    they land; 12 warm-up matmuls on zeroed tiles cover the ~7us engine
    boot and keep the PE HAM clock-gate at 2.4GHz when real work starts.
  - value head: for chunks 0..7, h3.Wo runs on the slack VectorE as 8
    in-place multiply-accumulates; the 128-partition reduction is one fp32
    ones-vector matmul deferred into the next chunk's L0 (saves ~0.85us
    PE per chunk vs 8 bf16 N=512 matmuls). Values land in value_row
    [1, 4352], reshaped once via SBUF->SBUF DMA into valP [128, 34].
  - GAE: a handful of [128,16] VectorE ops + tensor_tensor_scan per
    segment half, split into two partition halves so the first runs
    hidden under chunks 6-8. disc/dl (elementwise scalings of `cont`)
    and all reversals/permutations are host-side input prep.
"""

import sys

sys.path.insert(0, "/opt/trn_rl_repo")

import numpy as np

T, B, D, H = 16, 2048, 2048, 1024
NCORES = 8
BC = B // NCORES  # 256 batch per core
TP1 = T + 1
TOT = TP1 * BC  # 4352 MLP rows per core
DISCOUNT, LAMBDA = 0.99, 0.95
P = 128
KD = D // P  # 16 k-tiles for layer 0
KH = H // P  # 8 k-tiles for layers 1,2,out
MH = H // P  # 8 m-tiles of hidden units
CHUNKS = [512] * 8 + [256]  # sum = 4352
SEG = TP1  # 17 values per batch segment
NWARM = 12  # warm-up matmuls

_NC_CACHE = None


def _build():
    import concourse.bacc as bacc
    import concourse.mybir as mybir
    from concourse.tile import TileContext

    F32 = mybir.dt.float32
    BF16 = mybir.dt.bfloat16
    ALU = mybir.AluOpType
    ACTF = mybir.ActivationFunctionType

    nc = bacc.Bacc(None, target_bir_lowering=False, debug=False)

    # statesT: chunk 0 as 4 p-major quarters [4, 128, 4, n]; other chunks
    # one [128, 16, n] p-major block each. Flattened to rows of 1024.
    statesT_h = nc.declare_dram_parameter("statesT", [D * TOT // 1024, 1024], BF16, isOutput=False)
    # W0: [8 parts, 128, 2, 1024] part/p-major; W1/W2: [128, 8, 1024] p-major.
    w0_h = nc.declare_dram_parameter("W0t", [D, H], BF16, isOutput=False)
    w1_h = nc.declare_dram_parameter("W1t", [H, H], BF16, isOutput=False)
    w2_h = nc.declare_dram_parameter("W2t", [H, H], BF16, isOutput=False)
    wo_h = nc.declare_dram_parameter("WoP", [P, KH], BF16, isOutput=False)
    wof_h = nc.declare_dram_parameter("WoPf", [P, KH], F32, isOutput=False)
    bias_h = nc.declare_dram_parameter("biasP", [P, 3 * MH], F32, isOutput=False)
    bo_h = nc.declare_dram_parameter("bo", [1, 1], F32, isOutput=False)
    gae_h = nc.declare_dram_parameter("gaeP", [P, 6 * T], F32, isOutput=False)
    ret_h = nc.declare_dram_parameter("retP", [P, 2 * T], F32, isOutput=True)
    val_h = nc.declare_dram_parameter("valP", [P, 2 * T], F32, isOutput=True)

    with TileContext(nc) as tc:
        with (
            tc.tile_pool(name="wpool", bufs=1) as wpool,
            tc.tile_pool(name="stpool", bufs=1) as stpool,
            tc.tile_pool(name="hpool", bufs=1) as hpool,
            tc.tile_pool(name="tmp", bufs=3) as tmppool,
            tc.tile_pool(name="accp", bufs=2) as accpool,
            tc.tile_pool(name="gae", bufs=1) as gaepool,
            tc.tile_pool(name="psA", bufs=5, space="PSUM") as psApool,
            tc.tile_pool(name="psV", bufs=2, space="PSUM") as psVpool,
            tc.tile_pool(name="psW", bufs=1, space="PSUM") as psWpool,
        ):
            # ---- PE warm-up on zeroed tiles (overlaps the first DMAs) ----
            zw = wpool.tile([P, P], BF16, name="zw", tag="zw")
            nc.vector.memset(zw[:], 0.0)
            zx = wpool.tile([P, 512], BF16, name="zx", tag="zx")
            nc.vector.memset(zx[:], 0.0)
            zp = psWpool.tile([P, 512], F32, name="zp", tag="zp")
            for _ in range(NWARM):
                nc.tensor.matmul(
                    zp[:], lhsT=zw[:], rhs=zx[:], start=True, stop=True,
                    skip_group_check=True,
                )

            # ---- weights / constants ----
            # ALL large loads go on ONE queue (sync) in exact consumption
            # order: the 16 SDMA engines round-robin between active queues,
            # so a second queue halves the first's bandwidth (v5 trace: W0
            # on the scalar queue straggled to ~31us and stalled L0).
            # biasP first: the first ELU needs it, and ELUs recycle PSUM.
            biasP = wpool.tile([P, 3 * MH], F32, name="biasP", tag="biasP")
            nc.sync.dma_start(out=biasP[:], in_=bias_h[:])
            # W0 as 8 separate 512KB piece-tiles (2 k-tiles each) so chunk-0
            # matmuls depend only on the piece they read, not the whole 4MB.
            w0p = [
                wpool.tile([P, 2 * H], BF16, name=f"w0p{q}", tag=f"w0p{q}")
                for q in range(KD // 2)
            ]

            def load_w0_piece(q):
                nc.sync.dma_start(out=w0p[q][:], in_=w0_h[q * 256 : (q + 1) * 256, :])

            def w0slice(k, m):
                return w0p[k // 2][:, (k % 2) * H + m * P : (k % 2) * H + (m + 1) * P]

            w1all = wpool.tile([P, KH * H], BF16, name="w1all", tag="w1all")
            w2all = wpool.tile([P, KH * H], BF16, name="w2all", tag="w2all")
            wall = (None, w1all, w2all)

            def load_w12():
                nc.sync.dma_start(out=w1all[:], in_=w1_h[:])
                nc.sync.dma_start(out=w2all[:], in_=w2_h[:])

            wosb = wpool.tile([P, KH], BF16, name="wosb", tag="wosb")
            wosbf = wpool.tile([P, KH], F32, name="wosbf", tag="wosbf")
            bosb = wpool.tile([1, 1], F32, name="bosb", tag="bosb")
            ones_sb = wpool.tile([P, 1], BF16, name="ones_sb", tag="ones_sb")
            nc.vector.memset(ones_sb[:], 1.0)
            gaesb = gaepool.tile([P, 6 * T], F32, name="gaesb", tag="gaesb")
            rewsb = gaesb[:, 0 : 2 * T]
            discsb = gaesb[:, 2 * T : 4 * T]
            dlsb = gaesb[:, 4 * T : 6 * T]

            def load_consts():
                # tiny; parallel on the otherwise-idle scalar HWDGE queue
                nc.scalar.dma_start(out=wosb[:], in_=wo_h[:])
                nc.scalar.dma_start(out=wosbf[:], in_=wof_h[:])
                nc.scalar.dma_start(out=bosb[:], in_=bo_h[:])
                nc.scalar.dma_start(out=gaesb[:], in_=gae_h[:])

            value_row = gaepool.tile([1, TOT], F32, name="value_row", tag="value_row")
            valP = gaepool.tile([P, 2 * SEG], F32, name="valPsb", tag="valPsb")
            dtt = gaepool.tile([P, 2 * T], F32, name="dtt", tag="dtt")
            adv = gaepool.tile([P, 2 * T], F32, name="adv", tag="adv")
            retP = gaepool.tile([P, 2 * T], F32, name="retP", tag="retP")

            ALUc = ALU

            def gae_half(p0, p1):
                # GAE for partitions [p0, p1): value_row cols [p0*34, p1*34).
                pp = slice(p0, p1)
                nc.sync.dma_start(
                    out=valP[pp, :], in_=value_row[0:1, p0 * 2 * SEG : p1 * 2 * SEG]
                )
                for s in range(2):
                    ss = slice(s * T, (s + 1) * T)
                    vnext = valP[pp, s * SEG : s * SEG + T]
                    vcur = valP[pp, s * SEG + 1 : s * SEG + 1 + T]
                    nc.vector.tensor_mul(dtt[pp, ss], discsb[pp, ss], vnext)
                    nc.vector.tensor_add(dtt[pp, ss], dtt[pp, ss], rewsb[pp, ss])
                    nc.vector.tensor_sub(dtt[pp, ss], dtt[pp, ss], vcur)
                    nc.vector.tensor_tensor_scan(
                        adv[pp, ss], dlsb[pp, ss], dtt[pp, ss], 0.0, ALUc.mult, ALUc.add
                    )
                    nc.vector.tensor_add(retP[pp, ss], adv[pp, ss], vcur)
                    nc.sync.dma_start(out=val_h[pp, ss], in_=vcur)
                nc.sync.dma_start(out=ret_h[pp, :], in_=retP[pp, :])

            # ---- streamed MLP over column chunks ----
            # Deferred value head: for chunks 0..7 the per-chunk h3.Wo
            # contraction runs on the (slack) VectorE as 8 in-place
            # multiply-accumulates; the 128-partition reduction is ONE fp32
            # ones-vector matmul, emitted mid-way through the NEXT chunk's
            # L0 so the DVE chain latency never stalls the PE. This replaces
            # 8 bf16 N=n matmuls per chunk (saves ~0.85us/chunk of PE).
            pending = []

            def flush_head():
                if not pending:
                    return
                acc, pc0, pn = pending.pop()
                pv = psVpool.tile([1, pn], F32, name="pv", tag="pv")
                nc.tensor.matmul(
                    pv[:], lhsT=ones_sb[:], rhs=acc[:],
                    start=True, stop=True, skip_group_check=True,
                )
                nc.vector.tensor_scalar_add(
                    value_row[0:1, pc0 : pc0 + pn], pv[:], bosb[0:1, 0:1]
                )

            def elu(psm, li, m, hout, n):
                # ELU(z+b) = min(exp(z+b)-1, relu(z+b))
                bcol = biasP[:, li * MH + m : li * MH + m + 1]
                e = tmppool.tile([P, n], F32, name="e", tag="e")
                nc.scalar.activation(e[:], psm[:], ACTF.Exp, bias=bcol)
                rl = tmppool.tile([P, n], F32, name="rl", tag="rl")
                nc.vector.tensor_scalar(rl[:], psm[:], bcol, 0.0, ALU.add, ALU.max)
                nc.vector.scalar_tensor_tensor(
                    hout[:, m * n : (m + 1) * n], e[:], 1.0, rl[:],
                    ALU.subtract, ALU.min,
                )

            c0 = 0
            row0 = 0
            for ci, n in enumerate(CHUNKS):
                nrows = KD * P * n // 1024  # 1024 (n=512) or 512 (n=256)
                if ci == 0:
                    # chunk 0: four separate quarter-tiles (4 k-tiles each),
                    # each interleaved with the matching W0 pieces so the
                    # single DMA queue delivers in exact consumption order.
                    stq = []
                    for qi in range(4):
                        sq = stpool.tile([P, 4 * n], BF16, name=f"st0q{qi}", tag=f"st0q{qi}")
                        hr = nrows // 4
                        nc.sync.dma_start(
                            out=sq[:],
                            in_=statesT_h[row0 + qi * hr : row0 + (qi + 1) * hr, :],
                        )
                        stq.append(sq)
                        load_w0_piece(2 * qi)
                        load_w0_piece(2 * qi + 1)
                    load_w12()
                    load_consts()

                    def st0slice(k):
                        return stq[k // 4][:, (k % 4) * n : (k % 4 + 1) * n]

                else:
                    st_all = stpool.tile([P, KD * n], BF16, name="st", tag="st", bufs=2)
                    nc.sync.dma_start(
                        out=st_all[:], in_=statesT_h[row0 : row0 + nrows, :]
                    )
                row0 += nrows

                hs = []
                for li, nk in ((0, KD), (1, KH), (2, KH)):
                    rhs_src = hs[-1] if li else None
                    hout = hpool.tile([P, MH * n], BF16, name=f"h{li}", tag=f"h{li}", bufs=2)
                    if li == 0 and ci == 0:
                        # pass A: k-outer for m 0..4, consuming DMA pieces
                        # as they arrive (PE never waits for the full 6MB).
                        # 5 m-tiles: consumption 4.3us/k-group >= delivery
                        # 3.9us/group, so the PE never outruns the DMA.
                        psms = [
                            psApool.tile([P, n], F32, name="psm", tag="psm")
                            for _ in range(5)
                        ]
                        for k in range(KD):
                            for mi, psm in enumerate(psms):
                                nc.tensor.matmul(
                                    psm[:], lhsT=w0slice(k, mi), rhs=st0slice(k),
                                    start=(k == 0), stop=(k == KD - 1),
                                    skip_group_check=True,
                                )
                        for mi, psm in enumerate(psms):
                            elu(psm, 0, mi, hout, n)
                        # pass B: m-outer for m 5..7 (everything resident now)
                        for m in range(5, MH):
                            psm = psApool.tile([P, n], F32, name="psm", tag="psm")
                            for k in range(KD):
                                nc.tensor.matmul(
                                    psm[:], lhsT=w0slice(k, m), rhs=st0slice(k),
                                    start=(k == 0), stop=(k == KD - 1),
                                    skip_group_check=True,
                                )
                            elu(psm, 0, m, hout, n)
                        hs.append(hout)
                        continue
                    for m in range(MH):
                        psm = psApool.tile([P, n], F32, name="psm", tag="psm")
                        for k in range(nk):
                            if li == 0:
                                lhsT = w0slice(k, m)
                                rhs = st_all[:, k * n : (k + 1) * n]
                            else:
                                lhsT = wall[li][:, k * H + m * P : k * H + (m + 1) * P]
                                rhs = rhs_src[:, k * n : (k + 1) * n]
                            nc.tensor.matmul(
                                psm[:], lhsT=lhsT, rhs=rhs,
                                start=(k == 0), stop=(k == nk - 1),
                                skip_group_check=True,
                            )
                        elu(psm, li, m, hout, n)
                        if li == 0 and m == 2:
                            flush_head()
                    hs.append(hout)

                if ci < len(CHUNKS) - 1:
                    # value head via DVE: acc = sum_k h3_k * wo_k (fp32
                    # chain; the last op writes bf16 so the ones-reduction
                    # matmul runs at bf16 rate, 213ns vs 853ns fp32).
                    acc = accpool.tile([P, n], F32, name="acc", tag="acc")
                    accb = accpool.tile([P, n], BF16, name="accb", tag="accb")
                    nc.vector.tensor_scalar_mul(acc[:], hs[2][:, 0:n], wosbf[:, 0:1])
                    for k in range(1, KH - 1):
                        nc.vector.scalar_tensor_tensor(
                            acc[:], hs[2][:, k * n : (k + 1) * n], wosbf[:, k : k + 1],
                            acc[:], ALU.mult, ALU.add,
                        )
                    nc.vector.scalar_tensor_tensor(
                        accb[:], hs[2][:, (KH - 1) * n : KH * n], wosbf[:, KH - 1 : KH],
                        acc[:], ALU.mult, ALU.add,
                    )
                    pending.append((accb, c0, n))
                else:
                    # last chunk: direct PE head (keeps the tail short)
                    pv = psVpool.tile([1, n], F32, name="pv", tag="pv")
                    for k in range(KH):
                        nc.tensor.matmul(
                            pv[:],
                            lhsT=wosb[:, k : k + 1],
                            rhs=hs[2][:, k * n : (k + 1) * n],
                            start=(k == 0),
                            stop=(k == KH - 1),
                            skip_group_check=True,
                        )
                    nc.vector.tensor_scalar_add(
                        value_row[0:1, c0 : c0 + n], pv[:], bosb[0:1, 0:1]
                    )
                c0 += n
                # GAE for partitions 0..63 (value_row cols < 2176) can run
                # once chunks 0..4's heads are written (head(4) flushes
                # during chunk 5's L0); it hides under chunks 6-8.
                if ci == 5:
                    gae_half(0, 64)
            gae_half(64, P)

    nc.compile()
    return nc


def _get_nc():
    global _NC_CACHE
    if _NC_CACHE is None:
        _NC_CACHE = _build()
    return _NC_CACHE


def _pack_pmajor(w, nk):
    # [nk*128, cols] -> p-major [128, nk, cols] flattened back to same shape
    cols = w.shape[1]
    return np.ascontiguousarray(
        w.reshape(nk, P, cols).transpose(1, 0, 2).reshape(nk * P, cols)
    )


def _make_in_maps(inputs):
    import ml_dtypes

    BF = ml_dtypes.bfloat16
    states = np.asarray(inputs["states"], dtype=np.float32)
    reward = np.asarray(inputs["reward"], dtype=np.float32)
    cont = np.asarray(inputs["cont"], dtype=np.float32)

    # Feature-major states, b-major columns with reversed time:
    # full[d, b, r] = states[16-r, b, d] in bf16.
    st_bf = states.astype(BF)
    full = np.ascontiguousarray(st_bf[::-1].transpose(2, 1, 0))  # [D, B, TP1]

    W0 = np.asarray(inputs["W0"], np.float32).astype(BF)
    W1 = np.asarray(inputs["W1"], np.float32).astype(BF)
    W2 = np.asarray(inputs["W2"], np.float32).astype(BF)
    # W0: [8 parts, 128, 2, 1024] part-major then p-major
    W0t = np.ascontiguousarray(
        W0.reshape(8, 2, P, H).transpose(0, 2, 1, 3).reshape(D, H)
    )
    W1t = _pack_pmajor(W1, KH)
    W2t = _pack_pmajor(W2, KH)
    WoP = np.ascontiguousarray(
        np.asarray(inputs["Wo"], np.float32).astype(BF).reshape(KH, P).T
    )
    WoPf = np.ascontiguousarray(WoP.astype(np.float32))
    b3 = np.stack(
        [np.asarray(inputs[k], np.float32) for k in ("b0", "b1", "b2")]
    )  # [3, 1024]
    biasP = np.ascontiguousarray(b3.reshape(3, MH, P).transpose(2, 0, 1).reshape(P, 3 * MH))
    bo = np.ascontiguousarray(np.asarray(inputs["bo"], np.float32).reshape(1, 1))

    in_maps = []
    for c in range(NCORES):
        sl = slice(c * BC, (c + 1) * BC)
        # statesT for this core: [D, 4352] b-major/rev-t columns, then
        # per chunk: halves x [128, 8|16, n] p-major, flattened.
        stT = full[:, sl, :].reshape(D, TOT)
        blocks = []
        c0 = 0
        for ci, n in enumerate(CHUNKS):
            blk = stT[:, c0 : c0 + n].reshape(KD, P, n)  # [k, p, n]
            ndma = 4 if ci == 0 else 1
            kk = KD // ndma
            blocks.append(
                np.ascontiguousarray(
                    blk.reshape(ndma, kk, P, n).transpose(0, 2, 1, 3)
                ).reshape(-1)
            )
            c0 += n
        statesT = np.concatenate(blocks).reshape(D * TOT // 1024, 1024)

        # rewP[p, s*16+j] = reward[15-j, 2p+s]; disc uses cont[16-j].
        rr = reward[::-1, sl]  # [T, BC], rr[j] = reward[15-j]
        cc = cont[1:][::-1, sl]  # [T, BC], cc[j] = cont[16-j]
        rewP = rr.T.reshape(P, 2 * T)
        discP = (DISCOUNT * cc).T.reshape(P, 2 * T)
        dlP = (DISCOUNT * LAMBDA * cc).T.reshape(P, 2 * T)
        gaeP = np.ascontiguousarray(np.concatenate([rewP, discP, dlP], axis=1))
        in_maps.append(
            {
                "statesT": statesT,
                "W0t": W0t,
                "W1t": W1t,
                "W2t": W2t,
                "WoP": WoP,
                "WoPf": WoPf,
                "biasP": biasP,
                "bo": bo,
                "gaeP": gaeP,
            }
        )
    return in_maps


def _run(inputs, trace=False):
    from concourse.bass_utils import run_bass_kernel_spmd

    nc = _get_nc()
    in_maps = _make_in_maps(inputs)
    bkr = run_bass_kernel_spmd(nc, in_maps, list(range(NCORES)), trace=trace)
    ret = np.empty((T, B), np.float32)
    val = np.empty((T, B), np.float32)
    for c in range(NCORES):
        sl = slice(c * BC, (c + 1) * BC)
        # retP[p, s*16+j] -> ret[15-j, 2p+s]
        rp = bkr.results[c]["retP"].reshape(P, 2, T)[:, :, ::-1]  # [p, s, t]
        vp = bkr.results[c]["valP"].reshape(P, 2, T)[:, :, ::-1]
        ret[:, sl] = rp.transpose(2, 0, 1).reshape(T, BC)
        val[:, sl] = vp.transpose(2, 0, 1).reshape(T, BC)
    return (ret, val), bkr


def kernel(**inputs):
    out, _ = _run(inputs, trace=False)
    return out
